# revision 19
# baseline (speedup 1.0000x reference)
"""CapsuleLayer (dynamic routing, 3 iterations) Trainium2 Bass kernel.

Problem (hardcoded):
    x: [64, 2048, 8] f32, W: [2048, 32, 8, 16] f32
    u_hat[b,o,i,k] = sum_d x[b,i,d] * W[i,o,d,k]
    3 rounds of routing-by-agreement over logits b[B,O,I], softmax over O.
    out v: [64, 32, 16] f32.

Sharding: data-parallel over batch across 8 cores (8 batch rows each), W
replicated. Everything on-chip per core:
  - u_hat computed once on PE via block-diag trick:
      per i-tile of 16: lhsT[(g,d),(g,b)] = x (block-diag), rhs[(g,d),(k,o)] = W
      -> u[(g,b), (k,o)] tiles, stored bf16 in SBUF (16 MiB).
  - round 0: s0 = (1/32) sum_i u_hat via a second accumulating matmul with
      lhsT = x-tile (no block diag) directly from x/W (fp32-exact in PSUM).
  - rounds 1,2: per batch of 8 tiles: vu = u*v (DVE, bf16 2x), agreement =
      k-tree-reduce (DVE), logits update, batched softmax over O (one ACT exp
      per batch + DVE row-sum + recip), cu = u*c (DVE), s += ones-matmul over
      i-partitions (PE).
  - squash + partition broadcast of v via PE ones-matmul.
Free-dim layout is (k, o): column = k*32 + o.

Schedule notes (what the tuning bought, 405us -> ~333us):
  - W is DMA'd 8 tiles per transfer (8 KiB/partition lines), 6 transfers in
    flight: the 16.8 MiB replicated-W stream is the pass-0 floor (~70us at
    the ~320 GB/s 16-queue aggregate).
  - xblk is built per 16-tile chunk into a 5-deep ring so the DVE build
    stays ahead of the PE's LDWEIGHTS.
  - Pass-0 PSUM->SBUF evacuation splits each 2-tile pair across ACT+DVE
    (~660ns wall/pair); u-matmuls and s0-accumulation matmuls are grouped
    per chunk (each u<->s0 acc-group toggle costs ~90ns of PE issue rate).
  - Rounds are DVE-bound at their stock-op floor (~115us each: vu 35 +
    k-tree 34 + eu 35 + softmax smalls; tensor_tensor bf16 caps at 2
    elem/cyc/partition). The loop is software-pipelined: batch j+1's
    vu/tree issues before batch j's z/rz/s8/eu so the DVE never stalls on
    ACT's exp. Final batches taper 16->8->4->2->2 to shrink the exposed
    eu -> s-matmul -> squash tail.
  - Routing logits are linear in the accumulated v sum (b2 = u.(v0+v1)),
    so no per-round logit tensor is stored; round 2 re-treduces u.(v0+v1).
  - All activations (Exp/Ln/Square/Copy) are pinned to the
    natural_log_exp_and_others table set: one ACT_TABLE_LOAD total instead
    of a ~2.6us ping-pong at every round boundary.
  - reciprocal_approx_fast (51-ULP NR) replaces bit-exact reciprocal in
    softmax 1/Z and squash.
"""

import numpy as np
import ml_dtypes

BF16 = ml_dtypes.bfloat16

B, I, D, O, K = 64, 2048, 8, 32, 16
NC_N = 8              # cores
BL = B // NC_N        # 8 batch rows per core
G = 16                # i's per tile
T = I // G            # 128 tiles
FREE = O * K          # 512, layout (k,o): col = k*32+o
EPS = 1e-7
BATCH = 16            # tiles per DVE instruction batch in routing rounds
WQ = 8                # W tiles per DMA (8KB per partition line)
WBUFS = 6             # W DMA ring depth (WQ*WBUFS tiles in flight)
ACT_COPY_OF_8 = 5     # of every 8 tile-pair copies, this many go to ACT

_CACHE = {}


def _pin_act_table_set():
    """Force every activation used here (Exp/Ln/Square/Copy/Identity) to
    resolve to the one table set that contains them all
    (natural_log_exp_and_others), so the kernel does a single ACT_TABLE_LOAD
    instead of ping-ponging between the exp and ln sets at every round
    boundary (~2.6us per switch, on the critical path)."""
    import functools
    import concourse.hw_specs as hw_specs
    import concourse.bacc as bacc
    import concourse.mybir as mybir

    if _CACHE.get("act_patched"):
        return
    ACTF = mybir.ActivationFunctionType
    orig = hw_specs.get_activation_tables
    keep = "natural_log_exp_and_others"
    strip = set()
    for nm in ("Exp", "Ln", "Log", "Square", "Copy", "Identity"):
        if hasattr(ACTF, nm):
            strip.add(getattr(ACTF, nm))

    @functools.cache
    def patched(arch):
        tabs = orig(arch)
        out = {}
        for name, fns in tabs.items():
            out[name] = set(fns) if name == keep else set(fns) - strip
        return out

    hw_specs.get_activation_tables = patched
    bacc.get_activation_tables = patched
    _CACHE["act_patched"] = True


def _build_bass():
    import concourse.bass as bass
    import concourse.bacc as bacc
    import concourse.mybir as mybir
    import concourse.tile as tile

    _pin_act_table_set()

    f32 = mybir.dt.float32
    bf16 = mybir.dt.bfloat16
    nc = bacc.Bacc()

    wd = nc.dram_tensor("w", [T // WQ, 128, WQ, FREE], bf16, kind="ExternalInput")
    xtd = nc.dram_tensor("xt", [128, T, BL], bf16, kind="ExternalInput")
    maskd = nc.dram_tensor("mask", [128, 128], bf16, kind="ExternalInput")
    onesd = nc.dram_tensor("ones", [128, BL], bf16, kind="ExternalInput")
    onestd = nc.dram_tensor("onest", [BL, 128], bf16, kind="ExternalInput")
    outd = nc.dram_tensor("out", [BL, FREE], f32, kind="ExternalOutput")

    AX = mybir.AxisListType
    ALU = mybir.AluOpType
    ACTF = mybir.ActivationFunctionType

    with tile.TileContext(nc) as tc:
        with (
            tc.tile_pool(name="const", bufs=1) as constp,
            tc.tile_pool(name="u16", bufs=1) as up,
            tc.tile_pool(name="vexp", bufs=1) as vexpp,
            tc.tile_pool(name="psum_s", bufs=1, space="PSUM") as psum_s,
            tc.tile_pool(name="psum_v", bufs=1, space="PSUM") as psum_v,
        ):
            eps_sb = constp.tile([128, 1], f32)
            xt_sb = constp.tile([128, T, BL], bf16)
            ones_sb = constp.tile([128, BL], bf16)
            onest_sb = constp.tile([BL, 128], bf16)

            u16 = up.tile([128, T, FREE], bf16)

            # ---------------- pass 0: u_hat + s0 ----------------
            # s0 shares the s_ps bank (dead before round 1's s_ps is live),
            # freeing a PSUM bank for a third u-pair buffer
            s0_ps = psum_s.tile([BL, FREE], f32, tag="s_ps")
            with (
                tc.tile_pool(name="xblk", bufs=5) as xblkp,
                tc.tile_pool(name="wt", bufs=WBUFS) as wtp,
                tc.tile_pool(name="psum_u", bufs=3, space="PSUM") as psum_u,
            ):
                # block-diag xblk[g*8+d, tt, g*8+b] = x[b, c*16+tt..., d]
                # built ON-CHIP per 16-tile chunk (ring of 4): broadcast-
                # expand xt over the 16 column groups, then multiply by a
                # [128,128] block-diagonal 0/1 mask
                xchunks = {}
                mask_sb = constp.tile([128, 128], bf16)
                nc.gpsimd.dma_start(xt_sb[:], xtd[:])
                nc.gpsimd.dma_start(mask_sb[:], maskd[:])
                nc.gpsimd.memset(eps_sb[:], EPS)
                nc.gpsimd.dma_start(ones_sb[:], onesd[:])
                nc.gpsimd.dma_start(onest_sb[:], onestd[:])

                def build_xblk(c):
                    sl = slice(16 * c, 16 * (c + 1))
                    xb = xblkp.tile([128, 16, 128], bf16)
                    nc.vector.tensor_copy(
                        xb[:].rearrange("p t (g b) -> p t g b", g=G),
                        xt_sb[:, sl, :].unsqueeze(2).broadcast_to(
                            [128, 16, G, BL]))
                    nc.vector.tensor_mul(
                        xb[:], xb[:],
                        mask_sb[:].unsqueeze(1).broadcast_to([128, 16, 128]))
                    xchunks[c] = xb

                build_xblk(0)
                build_xblk(1)
                build_xblk(2)
                npair = 0

                def s0_chunk(qq):
                    # s0 accumulated from the evacuated bf16 u16 tiles with a
                    # block-diag ones stationary (sum over the 16 g's per b).
                    # Scheduled 2 chunks behind the u-matmul stream so these
                    # fill PE slack while the next W chunk is still in DMA --
                    # the W-paced stream then only carries the u-matmuls.
                    for j in range(WQ):
                        t = WQ * qq + j
                        nc.tensor.matmul(
                            s0_ps[:], ones_sb[:], u16[:, t, :],
                            start=(t == 0), stop=(t == T - 1))

                qchunk = 16 // WQ  # q's per 16-tile xblk chunk
                for q in range(T // WQ):
                    c = q // qchunk + 3
                    if q % qchunk == qchunk - 1 and c < 8:
                        build_xblk(c)
                    if q >= 3:
                        s0_chunk(q - 3)
                    wt = wtp.tile([128, WQ, FREE], bf16)
                    nc.gpsimd.dma_start(wt[:], wd[q])
                    # all u-matmuls of the chunk first, then all s0
                    # accumulations: each u<->s0 accumulation-group toggle
                    # costs ~90ns of PE issue rate, so batch them
                    for jj in range(WQ // 2):
                        ut_ps = psum_u.tile([128, 2, FREE], f32)
                        for j2 in range(2):
                            j = 2 * jj + j2
                            t = WQ * q + j
                            nc.tensor.matmul(
                                ut_ps[:, j2, :],
                                xchunks[t // 16][:, t % 16, :], wt[:, j, :])
                        # PSUM -> SBUF bf16 cast: split the pair across ACT
                        # and DVE so it evacuates in ~660ns wall instead of a
                        # 1.1us single-engine copy (which was pacing the PE).
                        # Every 4th pair goes wholly to ACT so the DVE keeps
                        # headroom for the xblk builds.
                        tp = WQ * q + 2 * jj
                        if npair % 4 == 3:
                            nc.scalar.copy(u16[:, tp:tp + 2, :], ut_ps[:])
                        else:
                            nc.scalar.copy(u16[:, tp, :], ut_ps[:, 0, :])
                            nc.vector.tensor_copy(u16[:, tp + 1, :], ut_ps[:, 1, :])
                        npair += 1
                for qq in (T // WQ - 3, T // WQ - 2, T // WQ - 1):
                    s0_chunk(qq)

            # ---------------- squash + broadcast helpers ----------------
            with tc.tile_pool(name="sq", bufs=1) as sqp:

                def squash_and_bcast(s_ps, scale_const, rnd):
                    """v = squash(s_ps * scale_const); returns vexp1 [128,FREE]
                    (bf16, round-r broadcast weights) or DMAs fp32 v to outd
                    if rnd==2. For rnd==1 the broadcast weight is w = v0+v1
                    (routing logits are linear in the accumulated v sum, so no
                    per-round logit storage is needed)."""
                    last = rnd == 2
                    # sq2 = (s_ps*sc)^2 on ACT (Square), straight from PSUM --
                    # keeps the boundary chain on one queue
                    sq2 = sqp.tile([BL, O, K], f32, tag="sq2")
                    nc.scalar.activation(
                        sq2[:],
                        s_ps[:].rearrange("p (k o) -> p o k", o=O),
                        ACTF.Square, scale=float(scale_const))
                    s2 = sqp.tile([BL, O], f32, tag="s2")
                    nc.vector.reduce_sum(s2[:], sq2[:], axis=AX.X)
                    # rt = sqrt(s2+eps) = exp(0.5*ln(s2+eps)): Ln/Exp/Square/
                    # Copy share one ACT function table (Sqrt does not), so no
                    # ACT_TABLE_LOAD lands in the round-boundary chain
                    lns = sqp.tile([BL, O], f32, tag="lns")
                    nc.scalar.activation(lns[:], s2[:], ACTF.Ln, bias=eps_sb[:BL])
                    rt = sqp.tile([BL, O], f32, tag="rt")
                    nc.scalar.activation(rt[:], lns[:], ACTF.Exp, scale=0.5)
                    # den = (s2+1)*rt in one DVE op
                    den = sqp.tile([BL, O], f32, tag="den")
                    nc.vector.scalar_tensor_tensor(
                        den[:], s2[:], 1.0, rt[:], ALU.add, ALU.mult)
                    rden = sqp.tile([BL, O], f32, tag="rden")
                    nc.vector.reciprocal_approx_fast(rden[:], den[:])
                    # scl = (s2*sc)*rden so v = s_ps*scl folds the s-scaling
                    scl = sqp.tile([BL, O], f32, tag="scl")
                    nc.vector.scalar_tensor_tensor(
                        scl[:], s2[:], float(scale_const), rden[:],
                        ALU.mult, ALU.mult)
                    # v = s_ps * scl (broadcast over k), straight from PSUM
                    v = sqp.tile([BL, K, O], f32 if last else bf16,
                                 tag="vf" if last else f"v{rnd}")
                    nc.vector.tensor_mul(
                        v[:], s_ps[:].rearrange("p (k o) -> p k o", o=O),
                        scl[:].unsqueeze(1).broadcast_to([BL, K, O]))
                    if last:
                        nc.gpsimd.dma_start(outd[:], v[:].rearrange("p k o -> p (k o)"))
                        return None
                    w = v
                    if rnd == 1:
                        # round-2 logits = u . (v0+v1)
                        w = sqp.tile([BL, K, O], bf16, tag="w")
                        nc.vector.tensor_add(w[:], v[:], squash_and_bcast.v0[:])
                    squash_and_bcast.v0 = v
                    # replicate w to all 16 partition groups via PE
                    vrep_ps = psum_v.tile([128, FREE], f32, tag="vrep")
                    nc.tensor.matmul(
                        vrep_ps[:], onest_sb[:],
                        w[:].rearrange("p k o -> p (k o)"))
                    vexp1 = vexpp.tile([128, FREE], bf16, tag="vexp1")
                    nc.scalar.copy(vexp1[:], vrep_ps[:])
                    return vexp1

                vexp1 = squash_and_bcast(s0_ps, 1.0 / O, rnd=0)

                # ---------------- rounds 1, 2 ----------------
                with (
                    tc.tile_pool(name="rnd", bufs=2) as rp,
                    tc.tile_pool(name="tree", bufs=1) as treep,
                    tc.tile_pool(name="rnd2", bufs=2) as rp2,
                ):
                    # taper the final batches so the last eu->s-matmul->squash
                    # chain exposes only a couple tiles of serial tail
                    batches = [BATCH] * (T // BATCH - 1) + [8, 4, 2, 2]
                    starts = [sum(batches[:j]) for j in range(len(batches))]

                    for rnd in (1, 2):
                        s_ps = psum_s.tile([BL, FREE], f32, tag="s_ps")

                        def stage_a(bi):
                            """vu + k-tree + logits + exp for batch bi."""
                            tb, bc = starts[bi], batches[bi]
                            u_sl = u16[:, tb:tb + bc, :]
                            vu = rp.tile([128, BATCH, FREE], bf16, tag="vu")
                            vu = vu[:, 0:bc, :]
                            nc.vector.tensor_mul(
                                vu[:], u_sl,
                                vexp1[:].unsqueeze(1).broadcast_to(
                                    [128, bc, FREE]))
                            # k-tree reduce: in (k,o) layout the k-halves are
                            # contiguous column blocks, so every level is a 3D
                            # AP; t2/t3 overlay t1's low half (out == in0
                            # elementwise, identical strides -> no hazard)
                            t1 = treep.tile([128, BATCH, 8 * O], bf16, tag="t1")
                            t1 = t1[:, 0:bc, :]
                            nc.vector.tensor_add(
                                t1[:], vu[:, :, 0:8 * O], vu[:, :, 8 * O:16 * O])
                            t2 = t1[:, :, 0:4 * O]
                            nc.vector.tensor_add(
                                t2, t1[:, :, 0:4 * O], t1[:, :, 4 * O:8 * O])
                            t3 = t1[:, :, 0:2 * O]
                            nc.vector.tensor_add(
                                t3, t1[:, :, 0:2 * O], t1[:, :, 2 * O:4 * O])
                            # logits (round 1: u.v0; round 2: u.(v0+v1))
                            lg = rp2.tile([128, BATCH, O], bf16, tag="lg")
                            lg = lg[:, 0:bc, :]
                            nc.vector.tensor_add(
                                lg[:], t1[:, :, 0:O], t1[:, :, O:2 * O])
                            e8 = rp2.tile([128, BATCH, O], bf16, tag="e")
                            e8 = e8[:, 0:bc, :]
                            nc.scalar.activation(e8[:], lg[:], ACTF.Exp)
                            return e8

                        def stage_b(bi, e8):
                            """softmax denom + eu + s-matmuls for batch bi.
                            Issued after stage_a(bi+1) so the DVE has the next
                            batch's vu/tree to chew on while ACT runs exp."""
                            tb, bc = starts[bi], batches[bi]
                            u_sl = u16[:, tb:tb + bc, :]
                            z8 = rp2.tile([128, BATCH], f32, tag="z")
                            z8 = z8[:, 0:bc]
                            nc.vector.reduce_sum(z8[:], e8[:], axis=AX.X)
                            rz8 = rp2.tile([128, BATCH], f32, tag="rz")
                            rz8 = rz8[:, 0:bc]
                            nc.vector.reciprocal_approx_fast(rz8[:], z8[:])
                            # per-tile stationary S[:, j, :] = ones * rz[:, j]
                            s8 = rp2.tile([128, BATCH, BL], bf16, tag="s8")
                            s8 = s8[:, 0:bc, :]
                            nc.vector.tensor_mul(
                                s8[:],
                                ones_sb[:].unsqueeze(1).broadcast_to(
                                    [128, bc, BL]),
                                rz8[:].unsqueeze(2).broadcast_to(
                                    [128, bc, BL]))
                            # eu = u * e (unnormalized; 1/Z is in the
                            # stationary)
                            eu = rp.tile([128, BATCH, K, O], bf16, tag="vu")
                            eu = eu[:, 0:bc, :, :]
                            nc.vector.tensor_mul(
                                eu[:], u_sl.rearrange("p t (k o) -> p t k o", o=O),
                                e8[:].unsqueeze(2).broadcast_to(
                                    [128, bc, K, O]))
                            # s += sum_i (1/Z)*eu  (PE partition reduce)
                            for j in range(bc):
                                t = tb + j
                                nc.tensor.matmul(
                                    s_ps[:], s8[:, j, :],
                                    eu[:, j, :, :].rearrange("p k o -> p (k o)"),
                                    start=(t == 0), stop=(t == T - 1))

                        e8_prev = stage_a(0)
                        for bi in range(len(batches)):
                            e8_cur = e8_prev
                            if bi + 1 < len(batches):
                                e8_prev = stage_a(bi + 1)
                            stage_b(bi, e8_cur)
                        vexp1 = squash_and_bcast(s_ps, 1.0, rnd=rnd)
    nc.finalize()
    return nc


def _host_prep():
    """Core-independent input prep pieces."""
    ones = np.zeros((128, BL), dtype=BF16)
    for g in range(G):
        for b in range(BL):
            ones[g * 8 + b, b] = 1
    onest = np.ascontiguousarray(ones.T)
    mask = np.zeros((128, 128), dtype=BF16)
    for g in range(G):
        mask[g * 8:(g + 1) * 8, g * 8:(g + 1) * 8] = 1
    return ones, onest, mask


def kernel(x: np.ndarray, W: np.ndarray) -> np.ndarray:
    from concourse import bass_utils

    if "nc" not in _CACHE:
        _CACHE["nc"] = _build_bass()
        _CACHE["ones"], _CACHE["onest"], _CACHE["mask"] = _host_prep()
    nc = _CACHE["nc"]

    # W -> [T, (g,d), (k,o)] : w[t, g*8+d, k*32+o] = W[t*16+g, o, d, k]
    wr = (W.reshape(T, G, O, D, K).transpose(0, 1, 3, 4, 2)
          .reshape(T, 128, FREE).astype(BF16))
    # 4 tiles per DMA: [T//WQ, 128, WQ, FREE]
    wr4 = np.ascontiguousarray(
        wr.reshape(T // WQ, WQ, 128, FREE).transpose(0, 2, 1, 3))
    in_maps = []
    for c in range(NC_N):
        xl = x[c * BL:(c + 1) * BL]  # [8, 2048, 8]
        # xt[g*8+d, t, b] = xl[b, t*16+g, d]
        xt = np.ascontiguousarray(
            xl.reshape(BL, T, G, D).transpose(2, 3, 1, 0).reshape(128, T, BL)
        ).astype(BF16)
        in_maps.append({"w": wr4, "xt": xt, "mask": _CACHE["mask"],
                        "ones": _CACHE["ones"], "onest": _CACHE["onest"]})

    _CACHE["in_maps"] = in_maps
    res = bass_utils.run_bass_kernel_spmd(nc, in_maps, core_ids=list(range(NC_N)))
    out = np.empty((B, O, K), np.float32)
    for c in range(NC_N):
        v = res.results[c]["out"].reshape(BL, K, O)  # (k,o) cols
        out[c * BL:(c + 1) * BL] = v.transpose(0, 2, 1)
    return out



# revision 20
# speedup vs baseline: 1.1936x; 1.1936x over previous
"""CapsuleLayer (dynamic routing, 3 iterations) Trainium2 Bass kernel.

Problem (hardcoded):
    x: [64, 2048, 8] f32, W: [2048, 32, 8, 16] f32
    u_hat[b,o,i,k] = sum_d x[b,i,d] * W[i,o,d,k]
    3 rounds of routing-by-agreement over logits b[B,O,I], softmax over O.
    out v: [64, 32, 16] f32.

Sharding: data-parallel over batch across 8 cores (8 batch rows each), W
replicated. Everything on-chip per core:
  - u_hat computed once on PE via block-diag trick:
      per i-tile of 16: lhsT[(g,d),(g,b)] = x (block-diag), rhs[(g,d),(k,o)] = W
      -> u[(g,b), (k,o)] tiles, stored bf16 in SBUF (16 MiB).
  - round 0: s0 = (1/32) sum_i u_hat via a second accumulating matmul with
      lhsT = x-tile (no block diag) directly from x/W (fp32-exact in PSUM).
  - rounds 1,2: per batch of 8 tiles: vu = u*v (DVE, bf16 2x), agreement =
      k-tree-reduce (DVE), logits update, batched softmax over O (one ACT exp
      per batch + DVE row-sum + recip), cu = u*c (DVE), s += ones-matmul over
      i-partitions (PE).
  - squash + partition broadcast of v via PE ones-matmul.
Free-dim layout is (k, o): column = k*32 + o.

Schedule notes (what the tuning bought, 405us -> ~333us):
  - W is DMA'd 8 tiles per transfer (8 KiB/partition lines), 6 transfers in
    flight: the 16.8 MiB replicated-W stream is the pass-0 floor (~70us at
    the ~320 GB/s 16-queue aggregate).
  - xblk is built per 16-tile chunk into a 5-deep ring so the DVE build
    stays ahead of the PE's LDWEIGHTS.
  - Pass-0 PSUM->SBUF evacuation splits each 2-tile pair across ACT+DVE
    (~660ns wall/pair); u-matmuls and s0-accumulation matmuls are grouped
    per chunk (each u<->s0 acc-group toggle costs ~90ns of PE issue rate).
  - Rounds are DVE-bound at their stock-op floor (~115us each: vu 35 +
    k-tree 34 + eu 35 + softmax smalls; tensor_tensor bf16 caps at 2
    elem/cyc/partition). The loop is software-pipelined: batch j+1's
    vu/tree issues before batch j's z/rz/s8/eu so the DVE never stalls on
    ACT's exp. Final batches taper 16->8->4->2->2 to shrink the exposed
    eu -> s-matmul -> squash tail.
  - Routing logits are linear in the accumulated v sum (b2 = u.(v0+v1)),
    so no per-round logit tensor is stored; round 2 re-treduces u.(v0+v1).
  - All activations (Exp/Ln/Square/Copy) are pinned to the
    natural_log_exp_and_others table set: one ACT_TABLE_LOAD total instead
    of a ~2.6us ping-pong at every round boundary.
  - reciprocal_approx_fast (51-ULP NR) replaces bit-exact reciprocal in
    softmax 1/Z and squash.
"""

import numpy as np
import ml_dtypes

BF16 = ml_dtypes.bfloat16

B, I, D, O, K = 64, 2048, 8, 32, 16
NC_N = 8              # cores
BL = B // NC_N        # 8 batch rows per core
G = 16                # i's per tile
T = I // G            # 128 tiles
FREE = O * K          # 512, layout (k,o): col = k*32+o
EPS = 1e-7
BATCH = 16            # tiles per DVE instruction batch in routing rounds
WQ = 8                # W tiles per DMA (8KB per partition line)
WBUFS = 6             # W DMA ring depth (WQ*WBUFS tiles in flight)
ACT_COPY_OF_8 = 5     # of every 8 tile-pair copies, this many go to ACT

_CACHE = {}


def _pin_act_table_set():
    """Force every activation used here (Exp/Ln/Square/Copy/Identity) to
    resolve to the one table set that contains them all
    (natural_log_exp_and_others), so the kernel does a single ACT_TABLE_LOAD
    instead of ping-ponging between the exp and ln sets at every round
    boundary (~2.6us per switch, on the critical path)."""
    import functools
    import concourse.hw_specs as hw_specs
    import concourse.bacc as bacc
    import concourse.mybir as mybir

    if _CACHE.get("act_patched"):
        return
    ACTF = mybir.ActivationFunctionType
    orig = hw_specs.get_activation_tables
    keep = "natural_log_exp_and_others"
    strip = set()
    for nm in ("Exp", "Ln", "Log", "Square", "Copy", "Identity"):
        if hasattr(ACTF, nm):
            strip.add(getattr(ACTF, nm))

    @functools.cache
    def patched(arch):
        tabs = orig(arch)
        out = {}
        for name, fns in tabs.items():
            out[name] = set(fns) if name == keep else set(fns) - strip
        return out

    hw_specs.get_activation_tables = patched
    bacc.get_activation_tables = patched
    _CACHE["act_patched"] = True


def _build_bass():
    import concourse.bass as bass
    import concourse.bacc as bacc
    import concourse.mybir as mybir
    import concourse.tile as tile

    _pin_act_table_set()

    f32 = mybir.dt.float32
    bf16 = mybir.dt.bfloat16
    nc = bacc.Bacc()

    wd = nc.dram_tensor("w", [T // WQ, 128, WQ, FREE], bf16, kind="ExternalInput")
    xtd = nc.dram_tensor("xt", [128, T, BL], bf16, kind="ExternalInput")
    maskd = nc.dram_tensor("mask", [128, 128], bf16, kind="ExternalInput")
    onesd = nc.dram_tensor("ones", [128, BL], bf16, kind="ExternalInput")
    onestd = nc.dram_tensor("onest", [BL, 128], bf16, kind="ExternalInput")
    outd = nc.dram_tensor("out", [BL, FREE], f32, kind="ExternalOutput")

    AX = mybir.AxisListType
    ALU = mybir.AluOpType
    ACTF = mybir.ActivationFunctionType

    with tile.TileContext(nc) as tc:
        with (
            tc.tile_pool(name="const", bufs=1) as constp,
            tc.tile_pool(name="u16", bufs=1) as up,
            tc.tile_pool(name="vexp", bufs=1) as vexpp,
            tc.tile_pool(name="psum_s", bufs=1, space="PSUM") as psum_s,
            tc.tile_pool(name="psum_v", bufs=1, space="PSUM") as psum_v,
        ):
            eps_sb = constp.tile([128, 1], f32)
            xt_sb = constp.tile([128, T, BL], bf16)
            ones_sb = constp.tile([128, BL], bf16)
            onest_sb = constp.tile([BL, 128], bf16)

            u16 = up.tile([128, T, FREE], bf16)

            # ---------------- pass 0: u_hat + s0 ----------------
            # s0 shares the s_ps bank (dead before round 1's s_ps is live),
            # freeing a PSUM bank for a third u-pair buffer
            s0_ps = psum_s.tile([BL, FREE], f32, tag="s_ps")
            with (
                tc.tile_pool(name="xblk", bufs=5) as xblkp,
                tc.tile_pool(name="wt", bufs=WBUFS) as wtp,
                tc.tile_pool(name="psum_u", bufs=3, space="PSUM") as psum_u,
            ):
                # block-diag xblk[g*8+d, tt, g*8+b] = x[b, c*16+tt..., d]
                # built ON-CHIP per 16-tile chunk (ring of 4): broadcast-
                # expand xt over the 16 column groups, then multiply by a
                # [128,128] block-diagonal 0/1 mask
                xchunks = {}
                mask_sb = constp.tile([128, 128], bf16)
                nc.gpsimd.dma_start(xt_sb[:], xtd[:])
                nc.gpsimd.dma_start(mask_sb[:], maskd[:])
                nc.gpsimd.memset(eps_sb[:], EPS)
                nc.gpsimd.dma_start(ones_sb[:], onesd[:])
                nc.gpsimd.dma_start(onest_sb[:], onestd[:])

                def build_xblk(c):
                    sl = slice(16 * c, 16 * (c + 1))
                    xb = xblkp.tile([128, 16, 128], bf16)
                    nc.vector.tensor_copy(
                        xb[:].rearrange("p t (g b) -> p t g b", g=G),
                        xt_sb[:, sl, :].unsqueeze(2).broadcast_to(
                            [128, 16, G, BL]))
                    nc.vector.tensor_mul(
                        xb[:], xb[:],
                        mask_sb[:].unsqueeze(1).broadcast_to([128, 16, 128]))
                    xchunks[c] = xb

                build_xblk(0)
                build_xblk(1)
                build_xblk(2)
                npair = 0

                def s0_chunk(qq):
                    # s0 accumulated from the evacuated bf16 u16 tiles with a
                    # block-diag ones stationary (sum over the 16 g's per b).
                    # Scheduled 2 chunks behind the u-matmul stream so these
                    # fill PE slack while the next W chunk is still in DMA --
                    # the W-paced stream then only carries the u-matmuls.
                    for j in range(WQ):
                        t = WQ * qq + j
                        nc.tensor.matmul(
                            s0_ps[:], ones_sb[:], u16[:, t, :],
                            start=(t == 0), stop=(t == T - 1))

                qchunk = 16 // WQ  # q's per 16-tile xblk chunk
                for q in range(T // WQ):
                    c = q // qchunk + 3
                    if q % qchunk == qchunk - 1 and c < 8:
                        build_xblk(c)
                    if q >= 3:
                        s0_chunk(q - 3)
                    wt = wtp.tile([128, WQ, FREE], bf16)
                    nc.gpsimd.dma_start(wt[:], wd[q])
                    # all u-matmuls of the chunk first, then all s0
                    # accumulations: each u<->s0 accumulation-group toggle
                    # costs ~90ns of PE issue rate, so batch them
                    for jj in range(WQ // 2):
                        ut_ps = psum_u.tile([128, 2, FREE], f32)
                        for j2 in range(2):
                            j = 2 * jj + j2
                            t = WQ * q + j
                            nc.tensor.matmul(
                                ut_ps[:, j2, :],
                                xchunks[t // 16][:, t % 16, :], wt[:, j, :])
                        # PSUM -> SBUF bf16 cast: split the pair across ACT
                        # and DVE so it evacuates in ~660ns wall instead of a
                        # 1.1us single-engine copy (which was pacing the PE).
                        # Every 4th pair goes wholly to ACT so the DVE keeps
                        # headroom for the xblk builds.
                        tp = WQ * q + 2 * jj
                        if npair % 4 == 3:
                            nc.scalar.copy(u16[:, tp:tp + 2, :], ut_ps[:])
                        else:
                            nc.scalar.copy(u16[:, tp, :], ut_ps[:, 0, :])
                            nc.vector.tensor_copy(u16[:, tp + 1, :], ut_ps[:, 1, :])
                        npair += 1
                for qq in (T // WQ - 3, T // WQ - 2, T // WQ - 1):
                    s0_chunk(qq)

            # ---------------- squash + broadcast helpers ----------------
            with tc.tile_pool(name="sq", bufs=1) as sqp:
                vrep_ps = psum_v.tile([128, FREE], f32, tag="vrep")

                def squash_and_bcast(s_ps, scale_const, rnd):
                    """v = squash(s_ps * scale_const); returns vexp1 [128,FREE]
                    (bf16, round-r broadcast weights) or DMAs fp32 v to outd
                    if rnd==2. For rnd==1 the broadcast weight is w = v0+v1
                    (routing logits are linear in the accumulated v sum, so no
                    per-round logit storage is needed)."""
                    last = rnd == 2
                    # sq2 = (s_ps*sc)^2 on ACT (Square), straight from PSUM --
                    # keeps the boundary chain on one queue
                    sq2 = sqp.tile([BL, O, K], f32, tag="sq2")
                    nc.scalar.activation(
                        sq2[:],
                        s_ps[:].rearrange("p (k o) -> p o k", o=O),
                        ACTF.Square, scale=float(scale_const))
                    s2 = sqp.tile([BL, O], f32, tag="s2")
                    nc.vector.reduce_sum(s2[:], sq2[:], axis=AX.X)
                    # rt = sqrt(s2+eps) = exp(0.5*ln(s2+eps)): Ln/Exp/Square/
                    # Copy share one ACT function table (Sqrt does not), so no
                    # ACT_TABLE_LOAD lands in the round-boundary chain
                    lns = sqp.tile([BL, O], f32, tag="lns")
                    nc.scalar.activation(lns[:], s2[:], ACTF.Ln, bias=eps_sb[:BL])
                    rt = sqp.tile([BL, O], f32, tag="rt")
                    nc.scalar.activation(rt[:], lns[:], ACTF.Exp, scale=0.5)
                    # den = (s2+1)*rt in one DVE op
                    den = sqp.tile([BL, O], f32, tag="den")
                    nc.vector.scalar_tensor_tensor(
                        den[:], s2[:], 1.0, rt[:], ALU.add, ALU.mult)
                    rden = sqp.tile([BL, O], f32, tag="rden")
                    nc.vector.reciprocal_approx_fast(rden[:], den[:])
                    # scl = (s2*sc)*rden so v = s_ps*scl folds the s-scaling
                    scl = sqp.tile([BL, O], f32, tag="scl")
                    nc.vector.scalar_tensor_tensor(
                        scl[:], s2[:], float(scale_const), rden[:],
                        ALU.mult, ALU.mult)
                    # v = s_ps * scl (broadcast over k), straight from PSUM
                    v = sqp.tile([BL, K, O], f32 if last else bf16,
                                 tag="vf" if last else f"v{rnd}")
                    nc.vector.tensor_mul(
                        v[:], s_ps[:].rearrange("p (k o) -> p k o", o=O),
                        scl[:].unsqueeze(1).broadcast_to([BL, K, O]))
                    if last:
                        nc.gpsimd.dma_start(outd[:], v[:].rearrange("p k o -> p (k o)"))
                        return None
                    # replicate v to all 16 partition groups via PE; round-2
                    # logits need u.(v0+v1), so round 1 ACCUMULATES its v-rep
                    # onto round 0's in PSUM instead of adding v0+v1 first
                    nc.tensor.matmul(
                        vrep_ps[:], onest_sb[:],
                        v[:].rearrange("p k o -> p (k o)"),
                        start=(rnd == 0), stop=(rnd == 1))
                    vexp1 = vexpp.tile([128, FREE], bf16, tag=f"vexp{rnd}")
                    nc.scalar.copy(vexp1[:], vrep_ps[:])
                    return vexp1

                vexp1 = squash_and_bcast(s0_ps, 1.0 / O, rnd=0)

                # ---------------- rounds 1, 2 ----------------
                with (
                    tc.tile_pool(name="rnd", bufs=2) as rp,
                    tc.tile_pool(name="tree", bufs=1) as treep,
                    tc.tile_pool(name="rnd2", bufs=2) as rp2,
                ):
                    # taper the final batches so the last eu->s-matmul->squash
                    # chain exposes only a couple tiles of serial tail
                    batches = [BATCH] * (T // BATCH - 1) + [8, 4, 2, 2]
                    starts = [sum(batches[:j]) for j in range(len(batches))]

                    for rnd in (1, 2):
                        s_ps = psum_s.tile([BL, FREE], f32, tag="s_ps")

                        def stage_a(bi):
                            """vu + k-tree + logits + exp for batch bi."""
                            tb, bc = starts[bi], batches[bi]
                            u_sl = u16[:, tb:tb + bc, :]
                            vu = rp.tile([128, BATCH, FREE], bf16, tag="vu")
                            vu = vu[:, 0:bc, :]
                            nc.vector.tensor_mul(
                                vu[:], u_sl,
                                vexp1[:].unsqueeze(1).broadcast_to(
                                    [128, bc, FREE]))
                            # k-tree reduce: in (k,o) layout the k-halves are
                            # contiguous column blocks, so every level is a 3D
                            # AP; t2/t3 overlay t1's low half (out == in0
                            # elementwise, identical strides -> no hazard)
                            t1 = treep.tile([128, BATCH, 8 * O], bf16, tag="t1")
                            t1 = t1[:, 0:bc, :]
                            nc.vector.tensor_add(
                                t1[:], vu[:, :, 0:8 * O], vu[:, :, 8 * O:16 * O])
                            t2 = t1[:, :, 0:4 * O]
                            nc.vector.tensor_add(
                                t2, t1[:, :, 0:4 * O], t1[:, :, 4 * O:8 * O])
                            t3 = t1[:, :, 0:2 * O]
                            nc.vector.tensor_add(
                                t3, t1[:, :, 0:2 * O], t1[:, :, 2 * O:4 * O])
                            # logits (round 1: u.v0; round 2: u.(v0+v1))
                            lg = rp2.tile([128, BATCH, O], bf16, tag="lg")
                            lg = lg[:, 0:bc, :]
                            nc.vector.tensor_add(
                                lg[:], t1[:, :, 0:O], t1[:, :, O:2 * O])
                            e8 = rp2.tile([128, BATCH, O], bf16, tag="e")
                            e8 = e8[:, 0:bc, :]
                            nc.scalar.activation(e8[:], lg[:], ACTF.Exp)
                            return e8

                        def stage_b(bi, e8):
                            """softmax denom + eu + s-matmuls for batch bi.
                            Issued after stage_a(bi+1) so the DVE has the next
                            batch's vu/tree to chew on while ACT runs exp."""
                            tb, bc = starts[bi], batches[bi]
                            u_sl = u16[:, tb:tb + bc, :]
                            z8 = rp2.tile([128, BATCH], f32, tag="z")
                            z8 = z8[:, 0:bc]
                            nc.vector.reduce_sum(z8[:], e8[:], axis=AX.X)
                            rz8 = rp2.tile([128, BATCH], f32, tag="rz")
                            rz8 = rz8[:, 0:bc]
                            nc.vector.reciprocal_approx_fast(rz8[:], z8[:])
                            # per-tile stationary S[:, j, :] = ones * rz[:, j]
                            s8 = rp2.tile([128, BATCH, BL], bf16, tag="s8")
                            s8 = s8[:, 0:bc, :]
                            nc.vector.tensor_mul(
                                s8[:],
                                ones_sb[:].unsqueeze(1).broadcast_to(
                                    [128, bc, BL]),
                                rz8[:].unsqueeze(2).broadcast_to(
                                    [128, bc, BL]))
                            # eu = u * e (unnormalized; 1/Z is in the
                            # stationary)
                            eu = rp.tile([128, BATCH, K, O], bf16, tag="vu")
                            eu = eu[:, 0:bc, :, :]
                            nc.vector.tensor_mul(
                                eu[:], u_sl.rearrange("p t (k o) -> p t k o", o=O),
                                e8[:].unsqueeze(2).broadcast_to(
                                    [128, bc, K, O]))
                            # s += sum_i (1/Z)*eu  (PE partition reduce)
                            for j in range(bc):
                                t = tb + j
                                nc.tensor.matmul(
                                    s_ps[:], s8[:, j, :],
                                    eu[:, j, :, :].rearrange("p k o -> p (k o)"),
                                    start=(t == 0), stop=(t == T - 1))

                        e8_prev = stage_a(0)
                        for bi in range(len(batches)):
                            e8_cur = e8_prev
                            if bi + 1 < len(batches):
                                e8_prev = stage_a(bi + 1)
                            stage_b(bi, e8_cur)
                        vexp1 = squash_and_bcast(s_ps, 1.0, rnd=rnd)
    nc.finalize()
    return nc


def _host_prep():
    """Core-independent input prep pieces."""
    ones = np.zeros((128, BL), dtype=BF16)
    for g in range(G):
        for b in range(BL):
            ones[g * 8 + b, b] = 1
    onest = np.ascontiguousarray(ones.T)
    mask = np.zeros((128, 128), dtype=BF16)
    for g in range(G):
        mask[g * 8:(g + 1) * 8, g * 8:(g + 1) * 8] = 1
    return ones, onest, mask


def kernel(x: np.ndarray, W: np.ndarray) -> np.ndarray:
    from concourse import bass_utils

    if "nc" not in _CACHE:
        _CACHE["nc"] = _build_bass()
        _CACHE["ones"], _CACHE["onest"], _CACHE["mask"] = _host_prep()
    nc = _CACHE["nc"]

    # W -> [T, (g,d), (k,o)] : w[t, g*8+d, k*32+o] = W[t*16+g, o, d, k]
    wr = (W.reshape(T, G, O, D, K).transpose(0, 1, 3, 4, 2)
          .reshape(T, 128, FREE).astype(BF16))
    # 4 tiles per DMA: [T//WQ, 128, WQ, FREE]
    wr4 = np.ascontiguousarray(
        wr.reshape(T // WQ, WQ, 128, FREE).transpose(0, 2, 1, 3))
    in_maps = []
    for c in range(NC_N):
        xl = x[c * BL:(c + 1) * BL]  # [8, 2048, 8]
        # xt[g*8+d, t, b] = xl[b, t*16+g, d]
        xt = np.ascontiguousarray(
            xl.reshape(BL, T, G, D).transpose(2, 3, 1, 0).reshape(128, T, BL)
        ).astype(BF16)
        in_maps.append({"w": wr4, "xt": xt, "mask": _CACHE["mask"],
                        "ones": _CACHE["ones"], "onest": _CACHE["onest"]})

    _CACHE["in_maps"] = in_maps
    res = bass_utils.run_bass_kernel_spmd(nc, in_maps, core_ids=list(range(NC_N)))
    out = np.empty((B, O, K), np.float32)
    for c in range(NC_N):
        v = res.results[c]["out"].reshape(BL, K, O)  # (k,o) cols
        out[c * BL:(c + 1) * BL] = v.transpose(0, 2, 1)
    return out



# revision 21
# speedup vs baseline: 1.1950x; 1.0012x over previous
"""CapsuleLayer (dynamic routing, 3 iterations) Trainium2 Bass kernel.

Problem (hardcoded):
    x: [64, 2048, 8] f32, W: [2048, 32, 8, 16] f32
    u_hat[b,o,i,k] = sum_d x[b,i,d] * W[i,o,d,k]
    3 rounds of routing-by-agreement over logits b[B,O,I], softmax over O.
    out v: [64, 32, 16] f32.

Sharding: data-parallel over batch across 8 cores (8 batch rows each), W
replicated. Everything on-chip per core:
  - u_hat computed once on PE via block-diag trick:
      per i-tile of 16: lhsT[(g,d),(g,b)] = x (block-diag), rhs[(g,d),(k,o)] = W
      -> u[(g,b), (k,o)] tiles, stored bf16 in SBUF (16 MiB).
  - round 0: s0 = (1/32) sum_i u_hat via ones-matmuls over the evacuated
      bf16 u16 tiles, scheduled 3 chunks behind the W-paced u-matmul stream
      so they fill PE slack while the next W chunk is in flight.
  - rounds 1,2: per batch of 8 tiles: vu = u*v (DVE, bf16 2x), agreement =
      k-tree-reduce (DVE), logits update, batched softmax over O (one ACT exp
      per batch + DVE row-sum + recip), cu = u*c (DVE), s += ones-matmul over
      i-partitions (PE).
  - squash + partition broadcast of v via PE ones-matmul.
Free-dim layout is (k, o): column = k*32 + o.

Schedule notes (what the tuning bought, 405us -> ~333us):
  - W is DMA'd 8 tiles per transfer (8 KiB/partition lines), 6 transfers in
    flight: the 16.8 MiB replicated-W stream is the pass-0 floor (~70us at
    the ~320 GB/s 16-queue aggregate).
  - xblk is built per 16-tile chunk into a 5-deep ring so the DVE build
    stays ahead of the PE's LDWEIGHTS.
  - Pass-0 PSUM->SBUF evacuation splits each 2-tile pair across ACT+DVE
    (~660ns wall/pair).
  - Rounds are DVE-bound at their stock-op floor (~115us each: vu 35 +
    k-tree 34 + eu 35 + softmax smalls; tensor_tensor bf16 caps at 2
    elem/cyc/partition). The loop is software-pipelined: batch j+1's
    vu/tree issues before batch j's z/rz/s8/eu so the DVE never stalls on
    ACT's exp. Final batches taper 16->8->4->2->2 to shrink the exposed
    eu -> s-matmul -> squash tail.
  - Routing logits are linear in the accumulated v sum (b2 = u.(v0+v1)),
    so no per-round logit tensor is stored; round 1's v-broadcast matmul
    accumulates onto round 0's in PSUM, so round 2 treduces u.(v0+v1).
  - All activations (Exp/Ln/Square/Copy) are pinned to the
    natural_log_exp_and_others table set: one ACT_TABLE_LOAD total instead
    of a ~2.6us ping-pong at every round boundary.
  - reciprocal_approx_fast (51-ULP NR) replaces bit-exact reciprocal in
    softmax 1/Z and squash.
"""

import numpy as np
import ml_dtypes

BF16 = ml_dtypes.bfloat16

B, I, D, O, K = 64, 2048, 8, 32, 16
NC_N = 8              # cores
BL = B // NC_N        # 8 batch rows per core
G = 16                # i's per tile
T = I // G            # 128 tiles
FREE = O * K          # 512, layout (k,o): col = k*32+o
EPS = 1e-7
BATCH = 16            # tiles per DVE instruction batch in routing rounds
WQ = 8                # W tiles per DMA (8KB per partition line)
WBUFS = 6             # W DMA ring depth (WQ*WBUFS tiles in flight)
ACT_COPY_OF_8 = 5     # of every 8 tile-pair copies, this many go to ACT

_CACHE = {}


def _pin_act_table_set():
    """Force every activation used here (Exp/Ln/Square/Copy/Identity) to
    resolve to the one table set that contains them all
    (natural_log_exp_and_others), so the kernel does a single ACT_TABLE_LOAD
    instead of ping-ponging between the exp and ln sets at every round
    boundary (~2.6us per switch, on the critical path)."""
    import functools
    import concourse.hw_specs as hw_specs
    import concourse.bacc as bacc
    import concourse.mybir as mybir

    if _CACHE.get("act_patched"):
        return
    ACTF = mybir.ActivationFunctionType
    orig = hw_specs.get_activation_tables
    keep = "natural_log_exp_and_others"
    strip = set()
    for nm in ("Exp", "Ln", "Log", "Square", "Copy", "Identity"):
        if hasattr(ACTF, nm):
            strip.add(getattr(ACTF, nm))

    @functools.cache
    def patched(arch):
        tabs = orig(arch)
        out = {}
        for name, fns in tabs.items():
            out[name] = set(fns) if name == keep else set(fns) - strip
        return out

    hw_specs.get_activation_tables = patched
    bacc.get_activation_tables = patched
    _CACHE["act_patched"] = True


def _build_bass():
    import concourse.bass as bass
    import concourse.bacc as bacc
    import concourse.mybir as mybir
    import concourse.tile as tile

    _pin_act_table_set()

    f32 = mybir.dt.float32
    bf16 = mybir.dt.bfloat16
    nc = bacc.Bacc()

    wd = nc.dram_tensor("w", [T // WQ, 128, WQ, FREE], bf16, kind="ExternalInput")
    xtd = nc.dram_tensor("xt", [128, T, BL], bf16, kind="ExternalInput")
    maskd = nc.dram_tensor("mask", [128, 128], bf16, kind="ExternalInput")
    onesd = nc.dram_tensor("ones", [128, BL], bf16, kind="ExternalInput")
    onestd = nc.dram_tensor("onest", [BL, 128], bf16, kind="ExternalInput")
    outd = nc.dram_tensor("out", [BL, FREE], f32, kind="ExternalOutput")

    AX = mybir.AxisListType
    ALU = mybir.AluOpType
    ACTF = mybir.ActivationFunctionType

    with tile.TileContext(nc) as tc:
        with (
            tc.tile_pool(name="const", bufs=1) as constp,
            tc.tile_pool(name="u16", bufs=1) as up,
            tc.tile_pool(name="vexp", bufs=1) as vexpp,
            tc.tile_pool(name="psum_s", bufs=1, space="PSUM") as psum_s,
            tc.tile_pool(name="psum_v", bufs=1, space="PSUM") as psum_v,
        ):
            eps_sb = constp.tile([128, 1], f32)
            xt_sb = constp.tile([128, T, BL], bf16)
            ones_sb = constp.tile([128, BL], bf16)
            onest_sb = constp.tile([BL, 128], bf16)

            u16 = up.tile([128, T, FREE], bf16)

            # ---------------- pass 0: u_hat + s0 ----------------
            # s0 shares the s_ps bank (dead before round 1's s_ps is live),
            # freeing a PSUM bank for a third u-pair buffer
            s0_ps = psum_s.tile([BL, FREE], f32, tag="s_ps")
            with (
                tc.tile_pool(name="xblk", bufs=5) as xblkp,
                tc.tile_pool(name="wt", bufs=WBUFS) as wtp,
                tc.tile_pool(name="psum_u", bufs=3, space="PSUM") as psum_u,
            ):
                # block-diag xblk[g*8+d, tt, g*8+b] = x[b, c*16+tt..., d]
                # built ON-CHIP per 16-tile chunk (ring of 4): broadcast-
                # expand xt over the 16 column groups, then multiply by a
                # [128,128] block-diagonal 0/1 mask
                xchunks = {}
                mask_sb = constp.tile([128, 128], bf16)
                nc.gpsimd.dma_start(xt_sb[:], xtd[:])
                nc.gpsimd.dma_start(mask_sb[:], maskd[:])
                nc.gpsimd.memset(eps_sb[:], EPS)
                nc.gpsimd.dma_start(ones_sb[:], onesd[:])
                nc.gpsimd.dma_start(onest_sb[:], onestd[:])

                def build_xblk(c):
                    sl = slice(16 * c, 16 * (c + 1))
                    xb = xblkp.tile([128, 16, 128], bf16)
                    nc.vector.tensor_copy(
                        xb[:].rearrange("p t (g b) -> p t g b", g=G),
                        xt_sb[:, sl, :].unsqueeze(2).broadcast_to(
                            [128, 16, G, BL]))
                    nc.vector.tensor_mul(
                        xb[:], xb[:],
                        mask_sb[:].unsqueeze(1).broadcast_to([128, 16, 128]))
                    xchunks[c] = xb

                build_xblk(0)
                build_xblk(1)
                build_xblk(2)
                npair = 0

                def s0_chunk(qq):
                    # s0 accumulated from the evacuated bf16 u16 tiles with a
                    # block-diag ones stationary (sum over the 16 g's per b).
                    # Scheduled 2 chunks behind the u-matmul stream so these
                    # fill PE slack while the next W chunk is still in DMA --
                    # the W-paced stream then only carries the u-matmuls.
                    for j in range(WQ):
                        t = WQ * qq + j
                        nc.tensor.matmul(
                            s0_ps[:], ones_sb[:], u16[:, t, :],
                            start=(t == 0), stop=(t == T - 1))

                qchunk = 16 // WQ  # q's per 16-tile xblk chunk
                for q in range(T // WQ):
                    c = q // qchunk + 3
                    if q % qchunk == qchunk - 1 and c < 8:
                        build_xblk(c)
                    if q >= 3:
                        s0_chunk(q - 3)
                    wt = wtp.tile([128, WQ, FREE], bf16)
                    nc.gpsimd.dma_start(wt[:], wd[q])
                    # all u-matmuls of the chunk first, then all s0
                    # accumulations: each u<->s0 accumulation-group toggle
                    # costs ~90ns of PE issue rate, so batch them
                    for jj in range(WQ // 2):
                        ut_ps = psum_u.tile([128, 2, FREE], f32)
                        for j2 in range(2):
                            j = 2 * jj + j2
                            t = WQ * q + j
                            nc.tensor.matmul(
                                ut_ps[:, j2, :],
                                xchunks[t // 16][:, t % 16, :], wt[:, j, :])
                        # PSUM -> SBUF bf16 cast: split the pair across ACT
                        # and DVE so it evacuates in ~660ns wall instead of a
                        # 1.1us single-engine copy (which was pacing the PE).
                        # Every 4th pair goes wholly to ACT so the DVE keeps
                        # headroom for the xblk builds.
                        tp = WQ * q + 2 * jj
                        if npair % 4 == 3:
                            nc.scalar.copy(u16[:, tp:tp + 2, :], ut_ps[:])
                        else:
                            nc.scalar.copy(u16[:, tp, :], ut_ps[:, 0, :])
                            nc.vector.tensor_copy(u16[:, tp + 1, :], ut_ps[:, 1, :])
                        npair += 1
                for qq in (T // WQ - 3, T // WQ - 2, T // WQ - 1):
                    s0_chunk(qq)

            # ---------------- squash + broadcast helpers ----------------
            with tc.tile_pool(name="sq", bufs=1) as sqp:
                vrep_ps = psum_v.tile([128, FREE], f32, tag="vrep")

                def squash_and_bcast(s_ps, scale_const, rnd):
                    """v = squash(s_ps * scale_const); returns vexp1 [128,FREE]
                    (bf16, round-r broadcast weights) or DMAs fp32 v to outd
                    if rnd==2. For rnd==1 the broadcast weight is w = v0+v1
                    (routing logits are linear in the accumulated v sum, so no
                    per-round logit storage is needed)."""
                    last = rnd == 2
                    # sq2 = (s_ps*sc)^2 on ACT (Square), straight from PSUM --
                    # keeps the boundary chain on one queue
                    sq2 = sqp.tile([BL, O, K], f32, tag="sq2")
                    nc.scalar.activation(
                        sq2[:],
                        s_ps[:].rearrange("p (k o) -> p o k", o=O),
                        ACTF.Square, scale=float(scale_const))
                    s2 = sqp.tile([BL, O], f32, tag="s2")
                    nc.vector.reduce_sum(s2[:], sq2[:], axis=AX.X)
                    # rt = sqrt(s2+eps) = exp(0.5*ln(s2+eps)): Ln/Exp/Square/
                    # Copy share one ACT function table (Sqrt does not), so no
                    # ACT_TABLE_LOAD lands in the round-boundary chain
                    lns = sqp.tile([BL, O], f32, tag="lns")
                    nc.scalar.activation(lns[:], s2[:], ACTF.Ln, bias=eps_sb[:BL])
                    rt = sqp.tile([BL, O], f32, tag="rt")
                    nc.scalar.activation(rt[:], lns[:], ACTF.Exp, scale=0.5)
                    # den = (s2+1)*rt in one DVE op
                    den = sqp.tile([BL, O], f32, tag="den")
                    nc.vector.scalar_tensor_tensor(
                        den[:], s2[:], 1.0, rt[:], ALU.add, ALU.mult)
                    rden = sqp.tile([BL, O], f32, tag="rden")
                    nc.vector.reciprocal_approx_fast(rden[:], den[:])
                    # scl = (s2*sc)*rden so v = s_ps*scl folds the s-scaling
                    scl = sqp.tile([BL, O], f32, tag="scl")
                    nc.vector.scalar_tensor_tensor(
                        scl[:], s2[:], float(scale_const), rden[:],
                        ALU.mult, ALU.mult)
                    # v = s_ps * scl (broadcast over k), straight from PSUM
                    v = sqp.tile([BL, K, O], f32 if last else bf16,
                                 tag="vf" if last else f"v{rnd}")
                    nc.vector.tensor_mul(
                        v[:], s_ps[:].rearrange("p (k o) -> p k o", o=O),
                        scl[:].unsqueeze(1).broadcast_to([BL, K, O]))
                    if last:
                        nc.gpsimd.dma_start(outd[:], v[:].rearrange("p k o -> p (k o)"))
                        return None
                    # replicate v to all 16 partition groups via PE; round-2
                    # logits need u.(v0+v1), so round 1 ACCUMULATES its v-rep
                    # onto round 0's in PSUM instead of adding v0+v1 first
                    nc.tensor.matmul(
                        vrep_ps[:], onest_sb[:],
                        v[:].rearrange("p k o -> p (k o)"),
                        start=(rnd == 0), stop=(rnd == 1))
                    vexp1 = vexpp.tile([128, FREE], bf16, tag=f"vexp{rnd}")
                    nc.scalar.copy(vexp1[:], vrep_ps[:])
                    return vexp1

                vexp1 = squash_and_bcast(s0_ps, 1.0 / O, rnd=0)

                # ---------------- rounds 1, 2 ----------------
                with (
                    tc.tile_pool(name="rnd", bufs=2) as rp,
                    tc.tile_pool(name="tree", bufs=1) as treep,
                    tc.tile_pool(name="rnd2", bufs=2) as rp2,
                ):
                    # taper the final batches so the last eu->s-matmul->squash
                    # chain exposes only a couple tiles of serial tail
                    batches = [BATCH] * (T // BATCH - 1) + [8, 4, 2, 2]
                    starts = [sum(batches[:j]) for j in range(len(batches))]

                    for rnd in (1, 2):
                        s_ps = psum_s.tile([BL, FREE], f32, tag="s_ps")

                        def stage_a(bi):
                            """vu + k-tree + logits + exp for batch bi."""
                            tb, bc = starts[bi], batches[bi]
                            u_sl = u16[:, tb:tb + bc, :]
                            vu = rp.tile([128, BATCH, FREE], bf16, tag="vu")
                            vu = vu[:, 0:bc, :]
                            nc.vector.tensor_mul(
                                vu[:], u_sl,
                                vexp1[:].unsqueeze(1).broadcast_to(
                                    [128, bc, FREE]))
                            # k-tree reduce: in (k,o) layout the k-halves are
                            # contiguous column blocks, so every level is a 3D
                            # AP; t2/t3 overlay t1's low half (out == in0
                            # elementwise, identical strides -> no hazard)
                            t1 = treep.tile([128, BATCH, 8 * O], bf16, tag="t1")
                            t1 = t1[:, 0:bc, :]
                            nc.vector.tensor_add(
                                t1[:], vu[:, :, 0:8 * O], vu[:, :, 8 * O:16 * O])
                            t2 = t1[:, :, 0:4 * O]
                            nc.vector.tensor_add(
                                t2, t1[:, :, 0:4 * O], t1[:, :, 4 * O:8 * O])
                            t3 = t1[:, :, 0:2 * O]
                            nc.vector.tensor_add(
                                t3, t1[:, :, 0:2 * O], t1[:, :, 2 * O:4 * O])
                            # logits (round 1: u.v0; round 2: u.(v0+v1))
                            lg = rp2.tile([128, BATCH, O], bf16, tag="lg")
                            lg = lg[:, 0:bc, :]
                            nc.vector.tensor_add(
                                lg[:], t1[:, :, 0:O], t1[:, :, O:2 * O])
                            e8 = rp2.tile([128, BATCH, O], bf16, tag="e")
                            e8 = e8[:, 0:bc, :]
                            nc.scalar.activation(e8[:], lg[:], ACTF.Exp)
                            return e8

                        def stage_b(bi, e8):
                            """softmax denom + eu + s-matmuls for batch bi.
                            Issued after stage_a(bi+1) so the DVE has the next
                            batch's vu/tree to chew on while ACT runs exp."""
                            tb, bc = starts[bi], batches[bi]
                            u_sl = u16[:, tb:tb + bc, :]
                            z8 = rp2.tile([128, BATCH], f32, tag="z")
                            z8 = z8[:, 0:bc]
                            nc.vector.reduce_sum(z8[:], e8[:], axis=AX.X)
                            rz8 = rp2.tile([128, BATCH], f32, tag="rz")
                            rz8 = rz8[:, 0:bc]
                            nc.vector.reciprocal_approx_fast(rz8[:], z8[:])
                            # per-tile stationary S[:, j, :] = ones * rz[:, j]
                            s8 = rp2.tile([128, BATCH, BL], bf16, tag="s8")
                            s8 = s8[:, 0:bc, :]
                            nc.vector.tensor_mul(
                                s8[:],
                                ones_sb[:].unsqueeze(1).broadcast_to(
                                    [128, bc, BL]),
                                rz8[:].unsqueeze(2).broadcast_to(
                                    [128, bc, BL]))
                            # eu = u * e (unnormalized; 1/Z is in the
                            # stationary)
                            eu = rp.tile([128, BATCH, K, O], bf16, tag="vu")
                            eu = eu[:, 0:bc, :, :]
                            nc.vector.tensor_mul(
                                eu[:], u_sl.rearrange("p t (k o) -> p t k o", o=O),
                                e8[:].unsqueeze(2).broadcast_to(
                                    [128, bc, K, O]))
                            # s += sum_i (1/Z)*eu  (PE partition reduce)
                            for j in range(bc):
                                t = tb + j
                                nc.tensor.matmul(
                                    s_ps[:], s8[:, j, :],
                                    eu[:, j, :, :].rearrange("p k o -> p (k o)"),
                                    start=(t == 0), stop=(t == T - 1))

                        e8_prev = stage_a(0)
                        for bi in range(len(batches)):
                            e8_cur = e8_prev
                            if bi + 1 < len(batches):
                                e8_prev = stage_a(bi + 1)
                            stage_b(bi, e8_cur)
                        vexp1 = squash_and_bcast(s_ps, 1.0, rnd=rnd)
    nc.finalize()
    return nc


def _host_prep():
    """Core-independent input prep pieces."""
    ones = np.zeros((128, BL), dtype=BF16)
    for g in range(G):
        for b in range(BL):
            ones[g * 8 + b, b] = 1
    onest = np.ascontiguousarray(ones.T)
    mask = np.zeros((128, 128), dtype=BF16)
    for g in range(G):
        mask[g * 8:(g + 1) * 8, g * 8:(g + 1) * 8] = 1
    return ones, onest, mask


def kernel(x: np.ndarray, W: np.ndarray) -> np.ndarray:
    from concourse import bass_utils

    if "nc" not in _CACHE:
        _CACHE["nc"] = _build_bass()
        _CACHE["ones"], _CACHE["onest"], _CACHE["mask"] = _host_prep()
    nc = _CACHE["nc"]

    # W -> [T, (g,d), (k,o)] : w[t, g*8+d, k*32+o] = W[t*16+g, o, d, k]
    wr = (W.reshape(T, G, O, D, K).transpose(0, 1, 3, 4, 2)
          .reshape(T, 128, FREE).astype(BF16))
    # 4 tiles per DMA: [T//WQ, 128, WQ, FREE]
    wr4 = np.ascontiguousarray(
        wr.reshape(T // WQ, WQ, 128, FREE).transpose(0, 2, 1, 3))
    in_maps = []
    for c in range(NC_N):
        xl = x[c * BL:(c + 1) * BL]  # [8, 2048, 8]
        # xt[g*8+d, t, b] = xl[b, t*16+g, d]
        xt = np.ascontiguousarray(
            xl.reshape(BL, T, G, D).transpose(2, 3, 1, 0).reshape(128, T, BL)
        ).astype(BF16)
        in_maps.append({"w": wr4, "xt": xt, "mask": _CACHE["mask"],
                        "ones": _CACHE["ones"], "onest": _CACHE["onest"]})

    _CACHE["in_maps"] = in_maps
    res = bass_utils.run_bass_kernel_spmd(nc, in_maps, core_ids=list(range(NC_N)))
    out = np.empty((B, O, K), np.float32)
    for c in range(NC_N):
        v = res.results[c]["out"].reshape(BL, K, O)  # (k,o) cols
        out[c * BL:(c + 1) * BL] = v.transpose(0, 2, 1)
    return out



# revision 22
# speedup vs baseline: 1.1984x; 1.0029x over previous
"""CapsuleLayer (dynamic routing, 3 iterations) Trainium2 Bass kernel.

Problem (hardcoded):
    x: [64, 2048, 8] f32, W: [2048, 32, 8, 16] f32
    u_hat[b,o,i,k] = sum_d x[b,i,d] * W[i,o,d,k]
    3 rounds of routing-by-agreement over logits b[B,O,I], softmax over O.
    out v: [64, 32, 16] f32.

Sharding: data-parallel over batch across 8 cores (8 batch rows each), W
replicated. Everything on-chip per core:
  - u_hat computed once on PE via block-diag trick:
      per i-tile of 16: lhsT[(g,d),(g,b)] = x (block-diag), rhs[(g,d),(k,o)] = W
      -> u[(g,b), (k,o)] tiles, stored bf16 in SBUF (16 MiB).
  - round 0: s0 = (1/32) sum_i u_hat via a second accumulating matmul with
      lhsT = x-tile (no block diag) directly from x/W (fp32-exact in PSUM).
  - rounds 1,2: per batch of 8 tiles: vu = u*v (DVE, bf16 2x), agreement =
      k-tree-reduce (DVE), logits update, batched softmax over O (one ACT exp
      per batch + DVE row-sum + recip), cu = u*c (DVE), s += ones-matmul over
      i-partitions (PE).
  - squash + partition broadcast of v via PE ones-matmul.
Free-dim layout is (k, o): column = k*32 + o.

Schedule notes (what the tuning bought, 405us -> ~333us):
  - W is DMA'd 8 tiles per transfer (8 KiB/partition lines), 6 transfers in
    flight: the 16.8 MiB replicated-W stream is the pass-0 floor (~70us at
    the ~320 GB/s 16-queue aggregate).
  - xblk is built per 16-tile chunk into a 5-deep ring so the DVE build
    stays ahead of the PE's LDWEIGHTS.
  - Pass-0 PSUM->SBUF evacuation splits each 2-tile pair across ACT+DVE
    (~660ns wall/pair).
  - Rounds are DVE-bound at their stock-op floor (~115us each: vu 35 +
    k-tree 34 + eu 35 + softmax smalls; tensor_tensor bf16 caps at 2
    elem/cyc/partition). The loop is software-pipelined: batch j+1's
    vu/tree issues before batch j's z/rz/s8/eu so the DVE never stalls on
    ACT's exp. Final batches taper 16->8->4->2->2 to shrink the exposed
    eu -> s-matmul -> squash tail.
  - Routing logits are linear in the accumulated v sum (b2 = u.(v0+v1)),
    so no per-round logit tensor is stored; round 1's v-broadcast matmul
    accumulates onto round 0's in PSUM, so round 2 treduces u.(v0+v1).
  - All activations (Exp/Ln/Square/Copy) are pinned to the
    natural_log_exp_and_others table set: one ACT_TABLE_LOAD total instead
    of a ~2.6us ping-pong at every round boundary.
  - reciprocal_approx_fast (51-ULP NR) replaces bit-exact reciprocal in
    softmax 1/Z and squash.
"""

import numpy as np
import ml_dtypes

BF16 = ml_dtypes.bfloat16

B, I, D, O, K = 64, 2048, 8, 32, 16
NC_N = 8              # cores
BL = B // NC_N        # 8 batch rows per core
G = 16                # i's per tile
T = I // G            # 128 tiles
FREE = O * K          # 512, layout (k,o): col = k*32+o
EPS = 1e-7
BATCH = 16            # tiles per DVE instruction batch in routing rounds
WQ = 8                # W tiles per DMA (8KB per partition line)
WBUFS = 6             # W DMA ring depth (WQ*WBUFS tiles in flight)
ACT_COPY_OF_8 = 5     # of every 8 tile-pair copies, this many go to ACT

_CACHE = {}


def _pin_act_table_set():
    """Force every activation used here (Exp/Ln/Square/Copy/Identity) to
    resolve to the one table set that contains them all
    (natural_log_exp_and_others), so the kernel does a single ACT_TABLE_LOAD
    instead of ping-ponging between the exp and ln sets at every round
    boundary (~2.6us per switch, on the critical path)."""
    import functools
    import concourse.hw_specs as hw_specs
    import concourse.bacc as bacc
    import concourse.mybir as mybir

    if _CACHE.get("act_patched"):
        return
    ACTF = mybir.ActivationFunctionType
    orig = hw_specs.get_activation_tables
    keep = "natural_log_exp_and_others"
    strip = set()
    for nm in ("Exp", "Ln", "Log", "Square", "Copy", "Identity"):
        if hasattr(ACTF, nm):
            strip.add(getattr(ACTF, nm))

    @functools.cache
    def patched(arch):
        tabs = orig(arch)
        out = {}
        for name, fns in tabs.items():
            out[name] = set(fns) if name == keep else set(fns) - strip
        return out

    hw_specs.get_activation_tables = patched
    bacc.get_activation_tables = patched
    _CACHE["act_patched"] = True


def _build_bass():
    import concourse.bass as bass
    import concourse.bacc as bacc
    import concourse.mybir as mybir
    import concourse.tile as tile

    _pin_act_table_set()

    f32 = mybir.dt.float32
    bf16 = mybir.dt.bfloat16
    nc = bacc.Bacc()

    wd = nc.dram_tensor("w", [T // WQ, 128, WQ, FREE], bf16, kind="ExternalInput")
    xtd = nc.dram_tensor("xt", [128, T, BL], bf16, kind="ExternalInput")
    maskd = nc.dram_tensor("mask", [128, 128], bf16, kind="ExternalInput")
    onesd = nc.dram_tensor("ones", [128, BL], bf16, kind="ExternalInput")
    onestd = nc.dram_tensor("onest", [BL, 128], bf16, kind="ExternalInput")
    outd = nc.dram_tensor("out", [BL, FREE], f32, kind="ExternalOutput")

    AX = mybir.AxisListType
    ALU = mybir.AluOpType
    ACTF = mybir.ActivationFunctionType

    with tile.TileContext(nc) as tc:
        with (
            tc.tile_pool(name="const", bufs=1) as constp,
            tc.tile_pool(name="u16", bufs=1) as up,
            tc.tile_pool(name="vexp", bufs=1) as vexpp,
            tc.tile_pool(name="psum_s", bufs=1, space="PSUM") as psum_s,
            tc.tile_pool(name="psum_v", bufs=1, space="PSUM") as psum_v,
        ):
            eps_sb = constp.tile([128, 1], f32)
            xt_sb = constp.tile([128, T, BL], bf16)
            ones_sb = constp.tile([128, BL], bf16)
            onest_sb = constp.tile([BL, 128], bf16)

            u16 = up.tile([128, T, FREE], bf16)

            # ---------------- pass 0: u_hat + s0 ----------------
            # s0 shares the s_ps bank (dead before round 1's s_ps is live),
            # freeing a PSUM bank for a third u-pair buffer
            s0_ps = psum_s.tile([BL, FREE], f32, tag="s_ps")
            with (
                tc.tile_pool(name="xblk", bufs=5) as xblkp,
                tc.tile_pool(name="wt", bufs=WBUFS) as wtp,
                tc.tile_pool(name="psum_u", bufs=3, space="PSUM") as psum_u,
            ):
                # block-diag xblk[g*8+d, tt, g*8+b] = x[b, c*16+tt..., d]
                # built ON-CHIP per 16-tile chunk (ring of 4): broadcast-
                # expand xt over the 16 column groups, then multiply by a
                # [128,128] block-diagonal 0/1 mask
                xchunks = {}
                mask_sb = constp.tile([128, 128], bf16)
                nc.gpsimd.dma_start(xt_sb[:], xtd[:])
                nc.gpsimd.dma_start(mask_sb[:], maskd[:])
                nc.gpsimd.memset(eps_sb[:], EPS)
                nc.gpsimd.dma_start(ones_sb[:], onesd[:])
                nc.gpsimd.dma_start(onest_sb[:], onestd[:])

                def build_xblk(c):
                    sl = slice(16 * c, 16 * (c + 1))
                    xb = xblkp.tile([128, 16, 128], bf16)
                    nc.vector.tensor_copy(
                        xb[:].rearrange("p t (g b) -> p t g b", g=G),
                        xt_sb[:, sl, :].unsqueeze(2).broadcast_to(
                            [128, 16, G, BL]))
                    nc.vector.tensor_mul(
                        xb[:], xb[:],
                        mask_sb[:].unsqueeze(1).broadcast_to([128, 16, 128]))
                    xchunks[c] = xb

                build_xblk(0)
                build_xblk(1)
                build_xblk(2)
                npair = 0

                qchunk = 16 // WQ  # q's per 16-tile xblk chunk
                for q in range(T // WQ):
                    c = q // qchunk + 3
                    if q % qchunk == qchunk - 1 and c < 8:
                        build_xblk(c)
                    wt = wtp.tile([128, WQ, FREE], bf16)
                    nc.gpsimd.dma_start(wt[:], wd[q])
                    # all u-matmuls of the chunk first, then all s0
                    # accumulations: each u<->s0 accumulation-group toggle
                    # costs ~90ns of PE issue rate, so batch them
                    for jj in range(WQ // 2):
                        ut_ps = psum_u.tile([128, 2, FREE], f32)
                        for j2 in range(2):
                            j = 2 * jj + j2
                            t = WQ * q + j
                            nc.tensor.matmul(
                                ut_ps[:, j2, :],
                                xchunks[t // 16][:, t % 16, :], wt[:, j, :])
                        # PSUM -> SBUF bf16 cast: split the pair across ACT
                        # and DVE so it evacuates in ~660ns wall instead of a
                        # 1.1us single-engine copy (which was pacing the PE).
                        # Every 4th pair goes wholly to ACT so the DVE keeps
                        # headroom for the xblk builds.
                        tp = WQ * q + 2 * jj
                        if npair % 4 == 3:
                            nc.scalar.copy(u16[:, tp:tp + 2, :], ut_ps[:])
                        else:
                            nc.scalar.copy(u16[:, tp, :], ut_ps[:, 0, :])
                            nc.vector.tensor_copy(u16[:, tp + 1, :], ut_ps[:, 1, :])
                        npair += 1
                    for j in range(WQ):
                        t = WQ * q + j
                        # s0 accumulation straight from x,W (fp32-exact),
                        # grouped after the chunk's u-matmuls: each u<->s0
                        # acc-group toggle costs ~90ns of PE issue rate
                        nc.tensor.matmul(
                            s0_ps[:], xt_sb[:, t, :], wt[:, j, :],
                            start=(t == 0), stop=(t == T - 1),
                        )

            # ---------------- squash + broadcast helpers ----------------
            with tc.tile_pool(name="sq", bufs=1) as sqp:
                vrep_ps = psum_v.tile([128, FREE], f32, tag="vrep")

                def squash_and_bcast(s_ps, scale_const, rnd):
                    """v = squash(s_ps * scale_const); returns vexp1 [128,FREE]
                    (bf16, round-r broadcast weights) or DMAs fp32 v to outd
                    if rnd==2. For rnd==1 the broadcast weight is w = v0+v1
                    (routing logits are linear in the accumulated v sum, so no
                    per-round logit storage is needed)."""
                    last = rnd == 2
                    # sq2 = (s_ps*sc)^2 on ACT (Square), straight from PSUM --
                    # keeps the boundary chain on one queue
                    sq2 = sqp.tile([BL, O, K], f32, tag="sq2")
                    nc.scalar.activation(
                        sq2[:],
                        s_ps[:].rearrange("p (k o) -> p o k", o=O),
                        ACTF.Square, scale=float(scale_const))
                    s2 = sqp.tile([BL, O], f32, tag="s2")
                    nc.vector.reduce_sum(s2[:], sq2[:], axis=AX.X)
                    # rt = sqrt(s2+eps) = exp(0.5*ln(s2+eps)): Ln/Exp/Square/
                    # Copy share one ACT function table (Sqrt does not), so no
                    # ACT_TABLE_LOAD lands in the round-boundary chain
                    lns = sqp.tile([BL, O], f32, tag="lns")
                    nc.scalar.activation(lns[:], s2[:], ACTF.Ln, bias=eps_sb[:BL])
                    rt = sqp.tile([BL, O], f32, tag="rt")
                    nc.scalar.activation(rt[:], lns[:], ACTF.Exp, scale=0.5)
                    # den = (s2+1)*rt in one DVE op
                    den = sqp.tile([BL, O], f32, tag="den")
                    nc.vector.scalar_tensor_tensor(
                        den[:], s2[:], 1.0, rt[:], ALU.add, ALU.mult)
                    rden = sqp.tile([BL, O], f32, tag="rden")
                    nc.vector.reciprocal_approx_fast(rden[:], den[:])
                    # scl = (s2*sc)*rden so v = s_ps*scl folds the s-scaling
                    scl = sqp.tile([BL, O], f32, tag="scl")
                    nc.vector.scalar_tensor_tensor(
                        scl[:], s2[:], float(scale_const), rden[:],
                        ALU.mult, ALU.mult)
                    # v = s_ps * scl (broadcast over k), straight from PSUM
                    v = sqp.tile([BL, K, O], f32 if last else bf16,
                                 tag="vf" if last else f"v{rnd}")
                    nc.vector.tensor_mul(
                        v[:], s_ps[:].rearrange("p (k o) -> p k o", o=O),
                        scl[:].unsqueeze(1).broadcast_to([BL, K, O]))
                    if last:
                        nc.gpsimd.dma_start(outd[:], v[:].rearrange("p k o -> p (k o)"))
                        return None
                    # replicate v to all 16 partition groups via PE; round-2
                    # logits need u.(v0+v1), so round 1 ACCUMULATES its v-rep
                    # onto round 0's in PSUM instead of adding v0+v1 first
                    nc.tensor.matmul(
                        vrep_ps[:], onest_sb[:],
                        v[:].rearrange("p k o -> p (k o)"),
                        start=(rnd == 0), stop=(rnd == 1))
                    vexp1 = vexpp.tile([128, FREE], bf16, tag=f"vexp{rnd}")
                    nc.scalar.copy(vexp1[:], vrep_ps[:])
                    return vexp1

                vexp1 = squash_and_bcast(s0_ps, 1.0 / O, rnd=0)

                # ---------------- rounds 1, 2 ----------------
                with (
                    tc.tile_pool(name="rnd", bufs=2) as rp,
                    tc.tile_pool(name="tree", bufs=1) as treep,
                    tc.tile_pool(name="rnd2", bufs=2) as rp2,
                ):
                    # taper the final batches so the last eu->s-matmul->squash
                    # chain exposes only a couple tiles of serial tail
                    batches = [BATCH] * (T // BATCH - 1) + [8, 4, 2, 2]
                    starts = [sum(batches[:j]) for j in range(len(batches))]

                    for rnd in (1, 2):
                        s_ps = psum_s.tile([BL, FREE], f32, tag="s_ps")

                        def stage_a(bi):
                            """vu + k-tree + logits + exp for batch bi."""
                            tb, bc = starts[bi], batches[bi]
                            u_sl = u16[:, tb:tb + bc, :]
                            vu = rp.tile([128, BATCH, FREE], bf16, tag="vu")
                            vu = vu[:, 0:bc, :]
                            nc.vector.tensor_mul(
                                vu[:], u_sl,
                                vexp1[:].unsqueeze(1).broadcast_to(
                                    [128, bc, FREE]))
                            # k-tree reduce: in (k,o) layout the k-halves are
                            # contiguous column blocks, so every level is a 3D
                            # AP; t2/t3 overlay t1's low half (out == in0
                            # elementwise, identical strides -> no hazard)
                            t1 = treep.tile([128, BATCH, 8 * O], bf16, tag="t1")
                            t1 = t1[:, 0:bc, :]
                            nc.vector.tensor_add(
                                t1[:], vu[:, :, 0:8 * O], vu[:, :, 8 * O:16 * O])
                            t2 = t1[:, :, 0:4 * O]
                            nc.vector.tensor_add(
                                t2, t1[:, :, 0:4 * O], t1[:, :, 4 * O:8 * O])
                            t3 = t1[:, :, 0:2 * O]
                            nc.vector.tensor_add(
                                t3, t1[:, :, 0:2 * O], t1[:, :, 2 * O:4 * O])
                            # logits (round 1: u.v0; round 2: u.(v0+v1))
                            lg = rp2.tile([128, BATCH, O], bf16, tag="lg")
                            lg = lg[:, 0:bc, :]
                            nc.vector.tensor_add(
                                lg[:], t1[:, :, 0:O], t1[:, :, O:2 * O])
                            e8 = rp2.tile([128, BATCH, O], bf16, tag="e")
                            e8 = e8[:, 0:bc, :]
                            nc.scalar.activation(e8[:], lg[:], ACTF.Exp)
                            return e8

                        def stage_b(bi, e8):
                            """softmax denom + eu + s-matmuls for batch bi.
                            Issued after stage_a(bi+1) so the DVE has the next
                            batch's vu/tree to chew on while ACT runs exp."""
                            tb, bc = starts[bi], batches[bi]
                            u_sl = u16[:, tb:tb + bc, :]
                            z8 = rp2.tile([128, BATCH], f32, tag="z")
                            z8 = z8[:, 0:bc]
                            nc.vector.reduce_sum(z8[:], e8[:], axis=AX.X)
                            rz8 = rp2.tile([128, BATCH], f32, tag="rz")
                            rz8 = rz8[:, 0:bc]
                            nc.vector.reciprocal_approx_fast(rz8[:], z8[:])
                            # per-tile stationary S[:, j, :] = ones * rz[:, j]
                            s8 = rp2.tile([128, BATCH, BL], bf16, tag="s8")
                            s8 = s8[:, 0:bc, :]
                            nc.vector.tensor_mul(
                                s8[:],
                                ones_sb[:].unsqueeze(1).broadcast_to(
                                    [128, bc, BL]),
                                rz8[:].unsqueeze(2).broadcast_to(
                                    [128, bc, BL]))
                            # eu = u * e (unnormalized; 1/Z is in the
                            # stationary)
                            eu = rp.tile([128, BATCH, K, O], bf16, tag="vu")
                            eu = eu[:, 0:bc, :, :]
                            nc.vector.tensor_mul(
                                eu[:], u_sl.rearrange("p t (k o) -> p t k o", o=O),
                                e8[:].unsqueeze(2).broadcast_to(
                                    [128, bc, K, O]))
                            # s += sum_i (1/Z)*eu  (PE partition reduce)
                            for j in range(bc):
                                t = tb + j
                                nc.tensor.matmul(
                                    s_ps[:], s8[:, j, :],
                                    eu[:, j, :, :].rearrange("p k o -> p (k o)"),
                                    start=(t == 0), stop=(t == T - 1))

                        e8_prev = stage_a(0)
                        for bi in range(len(batches)):
                            e8_cur = e8_prev
                            if bi + 1 < len(batches):
                                e8_prev = stage_a(bi + 1)
                            stage_b(bi, e8_cur)
                        vexp1 = squash_and_bcast(s_ps, 1.0, rnd=rnd)
    nc.finalize()
    return nc


def _host_prep():
    """Core-independent input prep pieces."""
    ones = np.zeros((128, BL), dtype=BF16)
    for g in range(G):
        for b in range(BL):
            ones[g * 8 + b, b] = 1
    onest = np.ascontiguousarray(ones.T)
    mask = np.zeros((128, 128), dtype=BF16)
    for g in range(G):
        mask[g * 8:(g + 1) * 8, g * 8:(g + 1) * 8] = 1
    return ones, onest, mask


def kernel(x: np.ndarray, W: np.ndarray) -> np.ndarray:
    from concourse import bass_utils

    if "nc" not in _CACHE:
        _CACHE["nc"] = _build_bass()
        _CACHE["ones"], _CACHE["onest"], _CACHE["mask"] = _host_prep()
    nc = _CACHE["nc"]

    # W -> [T, (g,d), (k,o)] : w[t, g*8+d, k*32+o] = W[t*16+g, o, d, k]
    wr = (W.reshape(T, G, O, D, K).transpose(0, 1, 3, 4, 2)
          .reshape(T, 128, FREE).astype(BF16))
    # 4 tiles per DMA: [T//WQ, 128, WQ, FREE]
    wr4 = np.ascontiguousarray(
        wr.reshape(T // WQ, WQ, 128, FREE).transpose(0, 2, 1, 3))
    in_maps = []
    for c in range(NC_N):
        xl = x[c * BL:(c + 1) * BL]  # [8, 2048, 8]
        # xt[g*8+d, t, b] = xl[b, t*16+g, d]
        xt = np.ascontiguousarray(
            xl.reshape(BL, T, G, D).transpose(2, 3, 1, 0).reshape(128, T, BL)
        ).astype(BF16)
        in_maps.append({"w": wr4, "xt": xt, "mask": _CACHE["mask"],
                        "ones": _CACHE["ones"], "onest": _CACHE["onest"]})

    _CACHE["in_maps"] = in_maps
    res = bass_utils.run_bass_kernel_spmd(nc, in_maps, core_ids=list(range(NC_N)))
    out = np.empty((B, O, K), np.float32)
    for c in range(NC_N):
        v = res.results[c]["out"].reshape(BL, K, O)  # (k,o) cols
        out[c * BL:(c + 1) * BL] = v.transpose(0, 2, 1)
    return out



# revision 23
# speedup vs baseline: 1.2046x; 1.0052x over previous
"""CapsuleLayer (dynamic routing, 3 iterations) Trainium2 Bass kernel.

Problem (hardcoded):
    x: [64, 2048, 8] f32, W: [2048, 32, 8, 16] f32
    u_hat[b,o,i,k] = sum_d x[b,i,d] * W[i,o,d,k]
    3 rounds of routing-by-agreement over logits b[B,O,I], softmax over O.
    out v: [64, 32, 16] f32.

Sharding: data-parallel over batch across 8 cores (8 batch rows each), W
replicated. Everything on-chip per core:
  - u_hat computed once on PE via block-diag trick:
      per i-tile of 16: lhsT[(g,d),(g,b)] = x (block-diag), rhs[(g,d),(k,o)] = W
      -> u[(g,b), (k,o)] tiles, stored bf16 in SBUF (16 MiB).
  - round 0: s0 = (1/32) sum_i u_hat via a second accumulating matmul with
      lhsT = x-tile (no block diag) directly from x/W (fp32-exact in PSUM).
  - rounds 1,2: per batch of 8 tiles: vu = u*v (DVE, bf16 2x), agreement =
      k-tree-reduce (DVE), logits update, batched softmax over O (one ACT exp
      per batch + DVE row-sum + recip), cu = u*c (DVE), s += ones-matmul over
      i-partitions (PE).
  - squash + partition broadcast of v via PE ones-matmul.
Free-dim layout is (k, o): column = k*32 + o.

Schedule notes (what the tuning bought, 405us -> ~333us):
  - W is DMA'd 8 tiles per transfer (8 KiB/partition lines), 6 transfers in
    flight: the 16.8 MiB replicated-W stream is the pass-0 floor (~70us at
    the ~320 GB/s 16-queue aggregate).
  - xblk is built per 16-tile chunk into a 5-deep ring so the DVE build
    stays ahead of the PE's LDWEIGHTS.
  - Pass-0 PSUM->SBUF evacuation splits each 2-tile pair across ACT+DVE
    (~660ns wall/pair).
  - Rounds are DVE-bound at their stock-op floor (~115us each: vu 35 +
    k-tree 34 + eu 35 + softmax smalls; tensor_tensor bf16 caps at 2
    elem/cyc/partition). The loop is software-pipelined: batch j+1's
    vu/tree issues before batch j's z/rz/s8/eu so the DVE never stalls on
    ACT's exp. Final batches taper 16->8->4->2->2 to shrink the exposed
    eu -> s-matmul -> squash tail.
  - Routing logits are linear in the accumulated v sum (b2 = u.(v0+v1)),
    so no per-round logit tensor is stored; round 1's v-broadcast matmul
    accumulates onto round 0's in PSUM, so round 2 treduces u.(v0+v1).
  - All activations (Exp/Ln/Square/Copy) are pinned to the
    natural_log_exp_and_others table set: one ACT_TABLE_LOAD total instead
    of a ~2.6us ping-pong at every round boundary.
  - reciprocal_approx_fast (51-ULP NR) replaces bit-exact reciprocal in
    softmax 1/Z and squash.
"""

import numpy as np
import ml_dtypes

BF16 = ml_dtypes.bfloat16

B, I, D, O, K = 64, 2048, 8, 32, 16
NC_N = 8              # cores
BL = B // NC_N        # 8 batch rows per core
G = 16                # i's per tile
T = I // G            # 128 tiles
FREE = O * K          # 512, layout (k,o): col = k*32+o
EPS = 1e-7
BATCH = 16            # tiles per DVE instruction batch in routing rounds
WQ = 8                # W tiles per DMA (8KB per partition line)
WBUFS = 6             # W DMA ring depth (WQ*WBUFS tiles in flight)
ACT_COPY_OF_8 = 5     # of every 8 tile-pair copies, this many go to ACT

_CACHE = {}


def _pin_act_table_set():
    """Force every activation used here (Exp/Ln/Square/Copy/Identity) to
    resolve to the one table set that contains them all
    (natural_log_exp_and_others), so the kernel does a single ACT_TABLE_LOAD
    instead of ping-ponging between the exp and ln sets at every round
    boundary (~2.6us per switch, on the critical path)."""
    import functools
    import concourse.hw_specs as hw_specs
    import concourse.bacc as bacc
    import concourse.mybir as mybir

    if _CACHE.get("act_patched"):
        return
    ACTF = mybir.ActivationFunctionType
    orig = hw_specs.get_activation_tables
    keep = "natural_log_exp_and_others"
    strip = set()
    for nm in ("Exp", "Ln", "Log", "Square", "Copy", "Identity"):
        if hasattr(ACTF, nm):
            strip.add(getattr(ACTF, nm))

    @functools.cache
    def patched(arch):
        tabs = orig(arch)
        out = {}
        for name, fns in tabs.items():
            out[name] = set(fns) if name == keep else set(fns) - strip
        return out

    hw_specs.get_activation_tables = patched
    bacc.get_activation_tables = patched
    _CACHE["act_patched"] = True


def _build_bass():
    import concourse.bass as bass
    import concourse.bacc as bacc
    import concourse.mybir as mybir
    import concourse.tile as tile

    _pin_act_table_set()

    f32 = mybir.dt.float32
    bf16 = mybir.dt.bfloat16
    nc = bacc.Bacc()

    wd = nc.dram_tensor("w", [T // WQ, 128, WQ, FREE], bf16, kind="ExternalInput")
    xtd = nc.dram_tensor("xt", [128, T, BL], bf16, kind="ExternalInput")
    maskd = nc.dram_tensor("mask", [128, 128], bf16, kind="ExternalInput")
    onesd = nc.dram_tensor("ones", [128, BL], bf16, kind="ExternalInput")
    onestd = nc.dram_tensor("onest", [BL, 128], bf16, kind="ExternalInput")
    outd = nc.dram_tensor("out", [BL, FREE], f32, kind="ExternalOutput")

    AX = mybir.AxisListType
    ALU = mybir.AluOpType
    ACTF = mybir.ActivationFunctionType

    with tile.TileContext(nc) as tc:
        with (
            tc.tile_pool(name="const", bufs=1) as constp,
            tc.tile_pool(name="u16", bufs=1) as up,
            tc.tile_pool(name="vexp", bufs=1) as vexpp,
            tc.tile_pool(name="psum_s", bufs=1, space="PSUM") as psum_s,
            tc.tile_pool(name="psum_v", bufs=1, space="PSUM") as psum_v,
        ):
            eps_sb = constp.tile([128, 1], f32)
            xt_sb = constp.tile([128, T, BL], bf16)
            ones_sb = constp.tile([128, BL], bf16)
            onest_sb = constp.tile([BL, 128], bf16)

            u16 = up.tile([128, T, FREE], bf16)

            # ---------------- pass 0: u_hat + s0 ----------------
            # s0 shares the s_ps bank (dead before round 1's s_ps is live),
            # freeing a PSUM bank for a third u-pair buffer
            s0_ps = psum_s.tile([BL, FREE], f32, tag="s_ps")
            with (
                tc.tile_pool(name="xblk", bufs=5) as xblkp,
                tc.tile_pool(name="wt", bufs=WBUFS) as wtp,
                tc.tile_pool(name="psum_u", bufs=3, space="PSUM") as psum_u,
            ):
                # block-diag xblk[g*8+d, tt, g*8+b] = x[b, c*16+tt..., d]
                # built ON-CHIP per 16-tile chunk (ring of 4): broadcast-
                # expand xt over the 16 column groups, then multiply by a
                # [128,128] block-diagonal 0/1 mask
                xchunks = {}
                mask_sb = constp.tile([128, 128], bf16)
                nc.gpsimd.dma_start(xt_sb[:], xtd[:])
                nc.gpsimd.dma_start(mask_sb[:], maskd[:])
                nc.gpsimd.memset(eps_sb[:], EPS)
                nc.gpsimd.dma_start(ones_sb[:], onesd[:])
                nc.gpsimd.dma_start(onest_sb[:], onestd[:])

                def build_xblk(c):
                    # xb[p, t, (g,b)] = mask[p, (g,b)] * xt[p, t, b] in ONE
                    # tensor_tensor op: both inputs broadcast-strided (mask
                    # over t, xt over g), saving a separate broadcast copy
                    sl = slice(16 * c, 16 * (c + 1))
                    xb = xblkp.tile([128, 16, 128], bf16)
                    nc.vector.tensor_mul(
                        xb[:].rearrange("p t (g b) -> p t g b", g=G),
                        mask_sb[:].rearrange("p (g b) -> p g b", g=G)
                        .unsqueeze(1).broadcast_to([128, 16, G, BL]),
                        xt_sb[:, sl, :].unsqueeze(2).broadcast_to(
                            [128, 16, G, BL]))
                    xchunks[c] = xb

                build_xblk(0)
                build_xblk(1)
                build_xblk(2)
                npair = 0

                qchunk = 16 // WQ  # q's per 16-tile xblk chunk
                for q in range(T // WQ):
                    c = q // qchunk + 3
                    if q % qchunk == qchunk - 1 and c < 8:
                        build_xblk(c)
                    wt = wtp.tile([128, WQ, FREE], bf16)
                    nc.gpsimd.dma_start(wt[:], wd[q])
                    # all u-matmuls of the chunk first, then all s0
                    # accumulations: each u<->s0 accumulation-group toggle
                    # costs ~90ns of PE issue rate, so batch them
                    for jj in range(WQ // 2):
                        ut_ps = psum_u.tile([128, 2, FREE], f32)
                        for j2 in range(2):
                            j = 2 * jj + j2
                            t = WQ * q + j
                            nc.tensor.matmul(
                                ut_ps[:, j2, :],
                                xchunks[t // 16][:, t % 16, :], wt[:, j, :])
                        # PSUM -> SBUF bf16 cast: split the pair across ACT
                        # and DVE so it evacuates in ~660ns wall instead of a
                        # 1.1us single-engine copy (which was pacing the PE).
                        # Every 4th pair goes wholly to ACT so the DVE keeps
                        # headroom for the xblk builds.
                        tp = WQ * q + 2 * jj
                        if npair % 4 == 3:
                            nc.scalar.copy(u16[:, tp:tp + 2, :], ut_ps[:])
                        else:
                            nc.scalar.copy(u16[:, tp, :], ut_ps[:, 0, :])
                            nc.vector.tensor_copy(u16[:, tp + 1, :], ut_ps[:, 1, :])
                        npair += 1
                    for j in range(WQ):
                        t = WQ * q + j
                        # s0 accumulation straight from x,W (fp32-exact),
                        # grouped after the chunk's u-matmuls: each u<->s0
                        # acc-group toggle costs ~90ns of PE issue rate
                        nc.tensor.matmul(
                            s0_ps[:], xt_sb[:, t, :], wt[:, j, :],
                            start=(t == 0), stop=(t == T - 1),
                        )

            # ---------------- squash + broadcast helpers ----------------
            with tc.tile_pool(name="sq", bufs=1) as sqp:
                vrep_ps = psum_v.tile([128, FREE], f32, tag="vrep")

                def squash_and_bcast(s_ps, scale_const, rnd):
                    """v = squash(s_ps * scale_const); returns vexp1 [128,FREE]
                    (bf16, round-r broadcast weights) or DMAs fp32 v to outd
                    if rnd==2. For rnd==1 the broadcast weight is w = v0+v1
                    (routing logits are linear in the accumulated v sum, so no
                    per-round logit storage is needed)."""
                    last = rnd == 2
                    # sq2 = (s_ps*sc)^2 on ACT (Square), straight from PSUM --
                    # keeps the boundary chain on one queue
                    sq2 = sqp.tile([BL, O, K], f32, tag="sq2")
                    nc.scalar.activation(
                        sq2[:],
                        s_ps[:].rearrange("p (k o) -> p o k", o=O),
                        ACTF.Square, scale=float(scale_const))
                    s2 = sqp.tile([BL, O], f32, tag="s2")
                    nc.vector.reduce_sum(s2[:], sq2[:], axis=AX.X)
                    # rt = sqrt(s2+eps) = exp(0.5*ln(s2+eps)): Ln/Exp/Square/
                    # Copy share one ACT function table (Sqrt does not), so no
                    # ACT_TABLE_LOAD lands in the round-boundary chain
                    lns = sqp.tile([BL, O], f32, tag="lns")
                    nc.scalar.activation(lns[:], s2[:], ACTF.Ln, bias=eps_sb[:BL])
                    rt = sqp.tile([BL, O], f32, tag="rt")
                    nc.scalar.activation(rt[:], lns[:], ACTF.Exp, scale=0.5)
                    # den = (s2+1)*rt in one DVE op
                    den = sqp.tile([BL, O], f32, tag="den")
                    nc.vector.scalar_tensor_tensor(
                        den[:], s2[:], 1.0, rt[:], ALU.add, ALU.mult)
                    rden = sqp.tile([BL, O], f32, tag="rden")
                    nc.vector.reciprocal_approx_fast(rden[:], den[:])
                    # scl = (s2*sc)*rden so v = s_ps*scl folds the s-scaling
                    scl = sqp.tile([BL, O], f32, tag="scl")
                    nc.vector.scalar_tensor_tensor(
                        scl[:], s2[:], float(scale_const), rden[:],
                        ALU.mult, ALU.mult)
                    # v = s_ps * scl (broadcast over k), straight from PSUM
                    v = sqp.tile([BL, K, O], f32 if last else bf16,
                                 tag="vf" if last else f"v{rnd}")
                    nc.vector.tensor_mul(
                        v[:], s_ps[:].rearrange("p (k o) -> p k o", o=O),
                        scl[:].unsqueeze(1).broadcast_to([BL, K, O]))
                    if last:
                        nc.gpsimd.dma_start(outd[:], v[:].rearrange("p k o -> p (k o)"))
                        return None
                    # replicate v to all 16 partition groups via PE; round-2
                    # logits need u.(v0+v1), so round 1 ACCUMULATES its v-rep
                    # onto round 0's in PSUM instead of adding v0+v1 first
                    nc.tensor.matmul(
                        vrep_ps[:], onest_sb[:],
                        v[:].rearrange("p k o -> p (k o)"),
                        start=(rnd == 0), stop=(rnd == 1))
                    vexp1 = vexpp.tile([128, FREE], bf16, tag=f"vexp{rnd}")
                    nc.scalar.copy(vexp1[:], vrep_ps[:])
                    return vexp1

                vexp1 = squash_and_bcast(s0_ps, 1.0 / O, rnd=0)

                # ---------------- rounds 1, 2 ----------------
                with (
                    tc.tile_pool(name="rnd", bufs=2) as rp,
                    tc.tile_pool(name="tree", bufs=1) as treep,
                    tc.tile_pool(name="rnd2", bufs=2) as rp2,
                ):
                    # taper the final batches so the last eu->s-matmul->squash
                    # chain exposes only a couple tiles of serial tail
                    batches = [BATCH] * (T // BATCH - 1) + [8, 4, 2, 2]
                    starts = [sum(batches[:j]) for j in range(len(batches))]

                    for rnd in (1, 2):
                        s_ps = psum_s.tile([BL, FREE], f32, tag="s_ps")

                        def stage_a(bi):
                            """vu + k-tree + logits + exp for batch bi."""
                            tb, bc = starts[bi], batches[bi]
                            u_sl = u16[:, tb:tb + bc, :]
                            vu = rp.tile([128, BATCH, FREE], bf16, tag="vu")
                            vu = vu[:, 0:bc, :]
                            nc.vector.tensor_mul(
                                vu[:], u_sl,
                                vexp1[:].unsqueeze(1).broadcast_to(
                                    [128, bc, FREE]))
                            # k-tree reduce: in (k,o) layout the k-halves are
                            # contiguous column blocks, so every level is a 3D
                            # AP; t2/t3 overlay t1's low half (out == in0
                            # elementwise, identical strides -> no hazard)
                            t1 = treep.tile([128, BATCH, 8 * O], bf16, tag="t1")
                            t1 = t1[:, 0:bc, :]
                            nc.vector.tensor_add(
                                t1[:], vu[:, :, 0:8 * O], vu[:, :, 8 * O:16 * O])
                            t2 = t1[:, :, 0:4 * O]
                            nc.vector.tensor_add(
                                t2, t1[:, :, 0:4 * O], t1[:, :, 4 * O:8 * O])
                            t3 = t1[:, :, 0:2 * O]
                            nc.vector.tensor_add(
                                t3, t1[:, :, 0:2 * O], t1[:, :, 2 * O:4 * O])
                            # logits (round 1: u.v0; round 2: u.(v0+v1))
                            lg = rp2.tile([128, BATCH, O], bf16, tag="lg")
                            lg = lg[:, 0:bc, :]
                            nc.vector.tensor_add(
                                lg[:], t1[:, :, 0:O], t1[:, :, O:2 * O])
                            e8 = rp2.tile([128, BATCH, O], bf16, tag="e")
                            e8 = e8[:, 0:bc, :]
                            nc.scalar.activation(e8[:], lg[:], ACTF.Exp)
                            return e8

                        def stage_b(bi, e8):
                            """softmax denom + eu + s-matmuls for batch bi.
                            Issued after stage_a(bi+1) so the DVE has the next
                            batch's vu/tree to chew on while ACT runs exp."""
                            tb, bc = starts[bi], batches[bi]
                            u_sl = u16[:, tb:tb + bc, :]
                            z8 = rp2.tile([128, BATCH], f32, tag="z")
                            z8 = z8[:, 0:bc]
                            nc.vector.reduce_sum(z8[:], e8[:], axis=AX.X)
                            rz8 = rp2.tile([128, BATCH], f32, tag="rz")
                            rz8 = rz8[:, 0:bc]
                            nc.vector.reciprocal_approx_fast(rz8[:], z8[:])
                            # per-tile stationary S[:, j, :] = ones * rz[:, j]
                            s8 = rp2.tile([128, BATCH, BL], bf16, tag="s8")
                            s8 = s8[:, 0:bc, :]
                            nc.vector.tensor_mul(
                                s8[:],
                                ones_sb[:].unsqueeze(1).broadcast_to(
                                    [128, bc, BL]),
                                rz8[:].unsqueeze(2).broadcast_to(
                                    [128, bc, BL]))
                            # eu = u * e (unnormalized; 1/Z is in the
                            # stationary)
                            eu = rp.tile([128, BATCH, K, O], bf16, tag="vu")
                            eu = eu[:, 0:bc, :, :]
                            nc.vector.tensor_mul(
                                eu[:], u_sl.rearrange("p t (k o) -> p t k o", o=O),
                                e8[:].unsqueeze(2).broadcast_to(
                                    [128, bc, K, O]))
                            # s += sum_i (1/Z)*eu  (PE partition reduce)
                            for j in range(bc):
                                t = tb + j
                                nc.tensor.matmul(
                                    s_ps[:], s8[:, j, :],
                                    eu[:, j, :, :].rearrange("p k o -> p (k o)"),
                                    start=(t == 0), stop=(t == T - 1))

                        e8_prev = stage_a(0)
                        for bi in range(len(batches)):
                            e8_cur = e8_prev
                            if bi + 1 < len(batches):
                                e8_prev = stage_a(bi + 1)
                            stage_b(bi, e8_cur)
                        vexp1 = squash_and_bcast(s_ps, 1.0, rnd=rnd)
    nc.finalize()
    return nc


def _host_prep():
    """Core-independent input prep pieces."""
    ones = np.zeros((128, BL), dtype=BF16)
    for g in range(G):
        for b in range(BL):
            ones[g * 8 + b, b] = 1
    onest = np.ascontiguousarray(ones.T)
    mask = np.zeros((128, 128), dtype=BF16)
    for g in range(G):
        mask[g * 8:(g + 1) * 8, g * 8:(g + 1) * 8] = 1
    return ones, onest, mask


def kernel(x: np.ndarray, W: np.ndarray) -> np.ndarray:
    from concourse import bass_utils

    if "nc" not in _CACHE:
        _CACHE["nc"] = _build_bass()
        _CACHE["ones"], _CACHE["onest"], _CACHE["mask"] = _host_prep()
    nc = _CACHE["nc"]

    # W -> [T, (g,d), (k,o)] : w[t, g*8+d, k*32+o] = W[t*16+g, o, d, k]
    wr = (W.reshape(T, G, O, D, K).transpose(0, 1, 3, 4, 2)
          .reshape(T, 128, FREE).astype(BF16))
    # 4 tiles per DMA: [T//WQ, 128, WQ, FREE]
    wr4 = np.ascontiguousarray(
        wr.reshape(T // WQ, WQ, 128, FREE).transpose(0, 2, 1, 3))
    in_maps = []
    for c in range(NC_N):
        xl = x[c * BL:(c + 1) * BL]  # [8, 2048, 8]
        # xt[g*8+d, t, b] = xl[b, t*16+g, d]
        xt = np.ascontiguousarray(
            xl.reshape(BL, T, G, D).transpose(2, 3, 1, 0).reshape(128, T, BL)
        ).astype(BF16)
        in_maps.append({"w": wr4, "xt": xt, "mask": _CACHE["mask"],
                        "ones": _CACHE["ones"], "onest": _CACHE["onest"]})

    _CACHE["in_maps"] = in_maps
    res = bass_utils.run_bass_kernel_spmd(nc, in_maps, core_ids=list(range(NC_N)))
    out = np.empty((B, O, K), np.float32)
    for c in range(NC_N):
        v = res.results[c]["out"].reshape(BL, K, O)  # (k,o) cols
        out[c * BL:(c + 1) * BL] = v.transpose(0, 2, 1)
    return out



# revision 24
# speedup vs baseline: 1.2263x; 1.0180x over previous
"""CapsuleLayer (dynamic routing, 3 iterations) Trainium2 Bass kernel.

Problem (hardcoded):
    x: [64, 2048, 8] f32, W: [2048, 32, 8, 16] f32
    u_hat[b,o,i,k] = sum_d x[b,i,d] * W[i,o,d,k]
    3 rounds of routing-by-agreement over logits b[B,O,I], softmax over O.
    out v: [64, 32, 16] f32.

Sharding: data-parallel over batch across 8 cores (8 batch rows each), W
replicated. Everything on-chip per core:
  - u_hat computed once on PE via block-diag trick:
      per i-tile of 16: lhsT[(g,d),(g,b)] = x (block-diag), rhs[(g,d),(k,o)] = W
      -> u[(g,b), (k,o)] tiles, stored bf16 in SBUF (16 MiB).
  - round 0: s0 = (1/32) sum_i u_hat via a second accumulating matmul with
      lhsT = x-tile (no block diag) directly from x/W (fp32-exact in PSUM).
  - rounds 1,2: per batch of 8 tiles: vu = u*v (DVE, bf16 2x), agreement =
      k-tree-reduce (DVE), logits update, batched softmax over O (one ACT exp
      per batch + DVE row-sum + recip), cu = u*c (DVE), s += ones-matmul over
      i-partitions (PE).
  - squash + partition broadcast of v via PE ones-matmul.
Free-dim layout is (k, o): column = k*32 + o.

Schedule notes (what the tuning bought, 405us -> ~333us):
  - W is DMA'd 8 tiles per transfer (8 KiB/partition lines), 6 transfers in
    flight: the 16.8 MiB replicated-W stream is the pass-0 floor (~70us at
    the ~320 GB/s 16-queue aggregate).
  - xblk is built per 16-tile chunk into a 5-deep ring so the DVE build
    stays ahead of the PE's LDWEIGHTS.
  - Pass-0 PSUM->SBUF evacuation splits each 2-tile pair across ACT+DVE
    (~660ns wall/pair).
  - Rounds are DVE-bound at their stock-op floor (~115us each: vu 35 +
    k-tree 34 + eu 35 + softmax smalls; tensor_tensor bf16 caps at 2
    elem/cyc/partition). The loop is software-pipelined: batch j+1's
    vu/tree issues before batch j's z/rz/s8/eu so the DVE never stalls on
    ACT's exp. Final batches taper 16->8->4->2->2 to shrink the exposed
    eu -> s-matmul -> squash tail.
  - Routing logits are linear in the accumulated v sum (b2 = u.(v0+v1)),
    so no per-round logit tensor is stored; round 1's v-broadcast matmul
    accumulates onto round 0's in PSUM, so round 2 treduces u.(v0+v1).
  - All activations (Exp/Ln/Square/Copy) are pinned to the
    natural_log_exp_and_others table set: one ACT_TABLE_LOAD total instead
    of a ~2.6us ping-pong at every round boundary.
  - reciprocal_approx_fast (51-ULP NR) replaces bit-exact reciprocal in
    softmax 1/Z and squash.
"""

import numpy as np
import ml_dtypes

BF16 = ml_dtypes.bfloat16

B, I, D, O, K = 64, 2048, 8, 32, 16
NC_N = 8              # cores
BL = B // NC_N        # 8 batch rows per core
G = 16                # i's per tile
T = I // G            # 128 tiles
FREE = O * K          # 512, layout (k,o): col = k*32+o
EPS = 1e-7
BATCH = 16            # tiles per DVE instruction batch in routing rounds
WQ = 8                # W tiles per DMA (8KB per partition line)
WBUFS = 6             # W DMA ring depth (WQ*WBUFS tiles in flight)
ACT_COPY_OF_8 = 5     # of every 8 tile-pair copies, this many go to ACT

_CACHE = {}


def _pin_act_table_set():
    """Force every activation used here (Exp/Ln/Square/Copy/Identity) to
    resolve to the one table set that contains them all
    (natural_log_exp_and_others), so the kernel does a single ACT_TABLE_LOAD
    instead of ping-ponging between the exp and ln sets at every round
    boundary (~2.6us per switch, on the critical path)."""
    import functools
    import concourse.hw_specs as hw_specs
    import concourse.bacc as bacc
    import concourse.mybir as mybir

    if _CACHE.get("act_patched"):
        return
    ACTF = mybir.ActivationFunctionType
    orig = hw_specs.get_activation_tables
    keep = "natural_log_exp_and_others"
    strip = set()
    for nm in ("Exp", "Ln", "Log", "Square", "Copy", "Identity"):
        if hasattr(ACTF, nm):
            strip.add(getattr(ACTF, nm))

    @functools.cache
    def patched(arch):
        tabs = orig(arch)
        out = {}
        for name, fns in tabs.items():
            out[name] = set(fns) if name == keep else set(fns) - strip
        return out

    hw_specs.get_activation_tables = patched
    bacc.get_activation_tables = patched
    _CACHE["act_patched"] = True


def _build_bass():
    import concourse.bass as bass
    import concourse.bacc as bacc
    import concourse.mybir as mybir
    import concourse.tile as tile

    _pin_act_table_set()

    f32 = mybir.dt.float32
    bf16 = mybir.dt.bfloat16
    nc = bacc.Bacc()

    wd = nc.dram_tensor("w", [T // WQ, 128, WQ, FREE], bf16, kind="ExternalInput")
    xtd = nc.dram_tensor("xt", [128, T, BL], bf16, kind="ExternalInput")
    maskd = nc.dram_tensor("mask", [128, 128], bf16, kind="ExternalInput")
    onesd = nc.dram_tensor("ones", [128, BL], bf16, kind="ExternalInput")
    onestd = nc.dram_tensor("onest", [BL, 128], bf16, kind="ExternalInput")
    outd = nc.dram_tensor("out", [BL, FREE], f32, kind="ExternalOutput")

    AX = mybir.AxisListType
    ALU = mybir.AluOpType
    ACTF = mybir.ActivationFunctionType

    with tile.TileContext(nc) as tc:
        with (
            tc.tile_pool(name="const", bufs=1) as constp,
            tc.tile_pool(name="u16", bufs=1) as up,
            tc.tile_pool(name="vexp", bufs=1) as vexpp,
            tc.tile_pool(name="psum_s", bufs=1, space="PSUM") as psum_s,
            tc.tile_pool(name="psum_v", bufs=1, space="PSUM") as psum_v,
        ):
            eps_sb = constp.tile([128, 1], f32)
            xt_sb = constp.tile([128, T, BL], bf16)
            ones_sb = constp.tile([128, BL], bf16)
            onest_sb = constp.tile([BL, 128], bf16)

            u16 = up.tile([128, T, FREE], bf16)

            # ---------------- pass 0: u_hat + s0 ----------------
            # s0 shares the s_ps bank (dead before round 1's s_ps is live),
            # freeing a PSUM bank for a third u-pair buffer
            s0_ps = psum_s.tile([BL, FREE], f32, tag="s_ps")
            with (
                tc.tile_pool(name="xblk", bufs=5) as xblkp,
                tc.tile_pool(name="wt", bufs=WBUFS) as wtp,
                tc.tile_pool(name="psum_u", bufs=6, space="PSUM") as psum_u,
            ):
                # block-diag xblk[g*8+d, tt, g*8+b] = x[b, c*16+tt..., d]
                # built ON-CHIP per 16-tile chunk (ring of 4): broadcast-
                # expand xt over the 16 column groups, then multiply by a
                # [128,128] block-diagonal 0/1 mask
                xchunks = {}
                mask_sb = constp.tile([128, 128], bf16)
                nc.gpsimd.dma_start(xt_sb[:], xtd[:])
                nc.gpsimd.dma_start(mask_sb[:], maskd[:])
                nc.gpsimd.memset(eps_sb[:], EPS)
                nc.gpsimd.dma_start(ones_sb[:], onesd[:])
                nc.gpsimd.dma_start(onest_sb[:], onestd[:])

                def build_xblk(c):
                    # xb[p, t, (g,b)] = mask[p, (g,b)] * xt[p, t, b] in ONE
                    # tensor_tensor op: both inputs broadcast-strided (mask
                    # over t, xt over g), saving a separate broadcast copy
                    sl = slice(16 * c, 16 * (c + 1))
                    xb = xblkp.tile([128, 16, 128], bf16)
                    nc.vector.tensor_mul(
                        xb[:].rearrange("p t (g b) -> p t g b", g=G),
                        mask_sb[:].rearrange("p (g b) -> p g b", g=G)
                        .unsqueeze(1).broadcast_to([128, 16, G, BL]),
                        xt_sb[:, sl, :].unsqueeze(2).broadcast_to(
                            [128, 16, G, BL]))
                    xchunks[c] = xb

                build_xblk(0)
                build_xblk(1)
                build_xblk(2)
                npair = 0

                qchunk = 16 // WQ  # q's per 16-tile xblk chunk
                for q in range(T // WQ):
                    c = q // qchunk + 3
                    if q % qchunk == qchunk - 1 and c < 8:
                        build_xblk(c)
                    wt = wtp.tile([128, WQ, FREE], bf16)
                    nc.gpsimd.dma_start(wt[:], wd[q])
                    # all u-matmuls of the chunk first, then all s0
                    # accumulations: each u<->s0 accumulation-group toggle
                    # costs ~90ns of PE issue rate, so batch them.
                    # Single-tile PSUM bufs (6 banks): each matmul decoupled
                    # from its neighbor's evacuation; copies alternate
                    # ACT/DVE per tile (ACT slightly more: DVE also builds
                    # xblk chunks).
                    for j in range(WQ):
                        t = WQ * q + j
                        ut_ps = psum_u.tile([128, FREE], f32)
                        nc.tensor.matmul(
                            ut_ps[:],
                            xchunks[t // 16][:, t % 16, :], wt[:, j, :])
                        if npair % 8 in (0, 2, 4, 6, 7):
                            nc.scalar.copy(u16[:, t, :], ut_ps[:])
                        else:
                            nc.vector.tensor_copy(u16[:, t, :], ut_ps[:])
                        npair += 1
                    for j in range(WQ):
                        t = WQ * q + j
                        # s0 accumulation straight from x,W (fp32-exact),
                        # grouped after the chunk's u-matmuls: each u<->s0
                        # acc-group toggle costs ~90ns of PE issue rate
                        nc.tensor.matmul(
                            s0_ps[:], xt_sb[:, t, :], wt[:, j, :],
                            start=(t == 0), stop=(t == T - 1),
                        )

            # ---------------- squash + broadcast helpers ----------------
            with tc.tile_pool(name="sq", bufs=1) as sqp:
                vrep_ps = psum_v.tile([128, FREE], f32, tag="vrep")

                def squash_and_bcast(s_ps, scale_const, rnd):
                    """v = squash(s_ps * scale_const); returns vexp1 [128,FREE]
                    (bf16, round-r broadcast weights) or DMAs fp32 v to outd
                    if rnd==2. For rnd==1 the broadcast weight is w = v0+v1
                    (routing logits are linear in the accumulated v sum, so no
                    per-round logit storage is needed)."""
                    last = rnd == 2
                    # sq2 = (s_ps*sc)^2 on ACT (Square), straight from PSUM --
                    # keeps the boundary chain on one queue
                    sq2 = sqp.tile([BL, O, K], f32, tag="sq2")
                    nc.scalar.activation(
                        sq2[:],
                        s_ps[:].rearrange("p (k o) -> p o k", o=O),
                        ACTF.Square, scale=float(scale_const))
                    s2 = sqp.tile([BL, O], f32, tag="s2")
                    nc.vector.reduce_sum(s2[:], sq2[:], axis=AX.X)
                    # rt = sqrt(s2+eps) = exp(0.5*ln(s2+eps)): Ln/Exp/Square/
                    # Copy share one ACT function table (Sqrt does not), so no
                    # ACT_TABLE_LOAD lands in the round-boundary chain
                    lns = sqp.tile([BL, O], f32, tag="lns")
                    nc.scalar.activation(lns[:], s2[:], ACTF.Ln, bias=eps_sb[:BL])
                    rt = sqp.tile([BL, O], f32, tag="rt")
                    nc.scalar.activation(rt[:], lns[:], ACTF.Exp, scale=0.5)
                    # den = (s2+1)*rt in one DVE op
                    den = sqp.tile([BL, O], f32, tag="den")
                    nc.vector.scalar_tensor_tensor(
                        den[:], s2[:], 1.0, rt[:], ALU.add, ALU.mult)
                    rden = sqp.tile([BL, O], f32, tag="rden")
                    nc.vector.reciprocal_approx_fast(rden[:], den[:])
                    # scl = (s2*sc)*rden so v = s_ps*scl folds the s-scaling
                    scl = sqp.tile([BL, O], f32, tag="scl")
                    nc.vector.scalar_tensor_tensor(
                        scl[:], s2[:], float(scale_const), rden[:],
                        ALU.mult, ALU.mult)
                    # v = s_ps * scl (broadcast over k), straight from PSUM
                    v = sqp.tile([BL, K, O], f32 if last else bf16,
                                 tag="vf" if last else f"v{rnd}")
                    nc.vector.tensor_mul(
                        v[:], s_ps[:].rearrange("p (k o) -> p k o", o=O),
                        scl[:].unsqueeze(1).broadcast_to([BL, K, O]))
                    if last:
                        nc.gpsimd.dma_start(outd[:], v[:].rearrange("p k o -> p (k o)"))
                        return None
                    # replicate v to all 16 partition groups via PE; round-2
                    # logits need u.(v0+v1), so round 1 ACCUMULATES its v-rep
                    # onto round 0's in PSUM instead of adding v0+v1 first
                    nc.tensor.matmul(
                        vrep_ps[:], onest_sb[:],
                        v[:].rearrange("p k o -> p (k o)"),
                        start=(rnd == 0), stop=(rnd == 1))
                    vexp1 = vexpp.tile([128, FREE], bf16, tag=f"vexp{rnd}")
                    nc.scalar.copy(vexp1[:], vrep_ps[:])
                    return vexp1

                vexp1 = squash_and_bcast(s0_ps, 1.0 / O, rnd=0)

                # ---------------- rounds 1, 2 ----------------
                with (
                    tc.tile_pool(name="rnd", bufs=2) as rp,
                    tc.tile_pool(name="tree", bufs=1) as treep,
                    tc.tile_pool(name="rnd2", bufs=2) as rp2,
                ):
                    # taper the final batches so the last eu->s-matmul->squash
                    # chain exposes only a couple tiles of serial tail
                    batches = [BATCH] * (T // BATCH - 1) + [8, 4, 2, 2]
                    starts = [sum(batches[:j]) for j in range(len(batches))]

                    for rnd in (1, 2):
                        s_ps = psum_s.tile([BL, FREE], f32, tag="s_ps")

                        def stage_a(bi):
                            """vu + k-tree + logits + exp for batch bi."""
                            tb, bc = starts[bi], batches[bi]
                            u_sl = u16[:, tb:tb + bc, :]
                            vu = rp.tile([128, BATCH, FREE], bf16, tag="vu")
                            vu = vu[:, 0:bc, :]
                            nc.vector.tensor_mul(
                                vu[:], u_sl,
                                vexp1[:].unsqueeze(1).broadcast_to(
                                    [128, bc, FREE]))
                            # k-tree reduce: in (k,o) layout the k-halves are
                            # contiguous column blocks, so every level is a 3D
                            # AP; t2/t3 overlay t1's low half (out == in0
                            # elementwise, identical strides -> no hazard)
                            t1 = treep.tile([128, BATCH, 8 * O], bf16, tag="t1")
                            t1 = t1[:, 0:bc, :]
                            nc.vector.tensor_add(
                                t1[:], vu[:, :, 0:8 * O], vu[:, :, 8 * O:16 * O])
                            t2 = t1[:, :, 0:4 * O]
                            nc.vector.tensor_add(
                                t2, t1[:, :, 0:4 * O], t1[:, :, 4 * O:8 * O])
                            t3 = t1[:, :, 0:2 * O]
                            nc.vector.tensor_add(
                                t3, t1[:, :, 0:2 * O], t1[:, :, 2 * O:4 * O])
                            # logits (round 1: u.v0; round 2: u.(v0+v1))
                            lg = rp2.tile([128, BATCH, O], bf16, tag="lg")
                            lg = lg[:, 0:bc, :]
                            nc.vector.tensor_add(
                                lg[:], t1[:, :, 0:O], t1[:, :, O:2 * O])
                            e8 = rp2.tile([128, BATCH, O], bf16, tag="e")
                            e8 = e8[:, 0:bc, :]
                            nc.scalar.activation(e8[:], lg[:], ACTF.Exp)
                            return e8

                        def stage_b(bi, e8):
                            """softmax denom + eu + s-matmuls for batch bi.
                            Issued after stage_a(bi+1) so the DVE has the next
                            batch's vu/tree to chew on while ACT runs exp."""
                            tb, bc = starts[bi], batches[bi]
                            u_sl = u16[:, tb:tb + bc, :]
                            z8 = rp2.tile([128, BATCH], f32, tag="z")
                            z8 = z8[:, 0:bc]
                            nc.vector.reduce_sum(z8[:], e8[:], axis=AX.X)
                            rz8 = rp2.tile([128, BATCH], f32, tag="rz")
                            rz8 = rz8[:, 0:bc]
                            nc.vector.reciprocal_approx_fast(rz8[:], z8[:])
                            # per-tile stationary S[:, j, :] = ones * rz[:, j]
                            s8 = rp2.tile([128, BATCH, BL], bf16, tag="s8")
                            s8 = s8[:, 0:bc, :]
                            nc.vector.tensor_mul(
                                s8[:],
                                ones_sb[:].unsqueeze(1).broadcast_to(
                                    [128, bc, BL]),
                                rz8[:].unsqueeze(2).broadcast_to(
                                    [128, bc, BL]))
                            # eu = u * e (unnormalized; 1/Z is in the
                            # stationary)
                            eu = rp.tile([128, BATCH, K, O], bf16, tag="vu")
                            eu = eu[:, 0:bc, :, :]
                            nc.vector.tensor_mul(
                                eu[:], u_sl.rearrange("p t (k o) -> p t k o", o=O),
                                e8[:].unsqueeze(2).broadcast_to(
                                    [128, bc, K, O]))
                            # s += sum_i (1/Z)*eu  (PE partition reduce)
                            for j in range(bc):
                                t = tb + j
                                nc.tensor.matmul(
                                    s_ps[:], s8[:, j, :],
                                    eu[:, j, :, :].rearrange("p k o -> p (k o)"),
                                    start=(t == 0), stop=(t == T - 1))

                        e8_prev = stage_a(0)
                        for bi in range(len(batches)):
                            e8_cur = e8_prev
                            if bi + 1 < len(batches):
                                e8_prev = stage_a(bi + 1)
                            stage_b(bi, e8_cur)
                        vexp1 = squash_and_bcast(s_ps, 1.0, rnd=rnd)
    nc.finalize()
    return nc


def _host_prep():
    """Core-independent input prep pieces."""
    ones = np.zeros((128, BL), dtype=BF16)
    for g in range(G):
        for b in range(BL):
            ones[g * 8 + b, b] = 1
    onest = np.ascontiguousarray(ones.T)
    mask = np.zeros((128, 128), dtype=BF16)
    for g in range(G):
        mask[g * 8:(g + 1) * 8, g * 8:(g + 1) * 8] = 1
    return ones, onest, mask


def kernel(x: np.ndarray, W: np.ndarray) -> np.ndarray:
    from concourse import bass_utils

    if "nc" not in _CACHE:
        _CACHE["nc"] = _build_bass()
        _CACHE["ones"], _CACHE["onest"], _CACHE["mask"] = _host_prep()
    nc = _CACHE["nc"]

    # W -> [T, (g,d), (k,o)] : w[t, g*8+d, k*32+o] = W[t*16+g, o, d, k]
    wr = (W.reshape(T, G, O, D, K).transpose(0, 1, 3, 4, 2)
          .reshape(T, 128, FREE).astype(BF16))
    # 4 tiles per DMA: [T//WQ, 128, WQ, FREE]
    wr4 = np.ascontiguousarray(
        wr.reshape(T // WQ, WQ, 128, FREE).transpose(0, 2, 1, 3))
    in_maps = []
    for c in range(NC_N):
        xl = x[c * BL:(c + 1) * BL]  # [8, 2048, 8]
        # xt[g*8+d, t, b] = xl[b, t*16+g, d]
        xt = np.ascontiguousarray(
            xl.reshape(BL, T, G, D).transpose(2, 3, 1, 0).reshape(128, T, BL)
        ).astype(BF16)
        in_maps.append({"w": wr4, "xt": xt, "mask": _CACHE["mask"],
                        "ones": _CACHE["ones"], "onest": _CACHE["onest"]})

    _CACHE["in_maps"] = in_maps
    res = bass_utils.run_bass_kernel_spmd(nc, in_maps, core_ids=list(range(NC_N)))
    out = np.empty((B, O, K), np.float32)
    for c in range(NC_N):
        v = res.results[c]["out"].reshape(BL, K, O)  # (k,o) cols
        out[c * BL:(c + 1) * BL] = v.transpose(0, 2, 1)
    return out

